# revision 46
# baseline (speedup 1.0000x reference)
"""AxialSelfAttention Trainium2 Bass kernel.

Reference computation (per batch b):
    xs  = x[b] reshaped [N=2048, E=512]
    qkv = xs @ W + bias                      # [N, 3E]
    q, k, v = split(qkv)
    row:  P = softmax(q @ k.T / sqrt(E));  out_row = P @ v
    col:  A = softmax(q.T @ k / sqrt(E));  out_col = v @ A.T
    out = out_row + out_col                  # [N, E]

Sharding: data-parallel over batch B=32 across 8 cores (4 batches/core).

Best variant (33, see build_nc_v24): St-direct row attention + fp8
DoubleRow.  Per batch:
  - x chunks are PE-transposed into rolling 512-token xT slices (f32r);
    q/k/v projections are f32r matmuls from xT (precision floor: bf16
    projections already fail the 2e-2 gate through the col path).
  - Natural q/k (bf16) are saved in SBUF; col-S runs as a burst of bf16
    matmuls in rotating PSUM banks, then max-subtracted softmax -> Acol,
    DMA-XBAR-transposed to AcolT.
  - Qt/Kt come from bf16 PE transposes of q/k, drained fp8 into one merged
    QKt tile (all 8 transposes of a chunk share one PSUM bank + one DVE
    drain).
  - Row attention computes S TRANSPOSED (stat=Kt chunk, mov=Qt slice) in
    fp8 DoubleRow; exp(St) lands directly in the PV stationary layout
    (fp8, unnormalized - row logits are in [-1.3, 1.0] so exp fits fp8).
    Row sums come from ones-vector DoubleRow matmuls; 1/rowsum is applied
    in the final DVE merge (out = po * rinv + out_col).
  - PV and colout accumulate in separate PSUM tiles; merged on DVE.
Measured on HW (8 cores, reps-slope): v13 baseline 1.10 ms -> v33 ~0.8 ms
per invocation; rel err 1.6e-3.
"""

import sys

for _p in ("/opt/trn_rl_repo", "/root/.axon_site/_ro/trn_rl_repo"):
    if _p not in sys.path:
        sys.path.append(_p)

import numpy as np

B, N, E = 32, 2048, 512
NCORES = 8
NB = B // NCORES  # batches per core
NE = N // 128  # 16 token chunks
ED = E // 128  # 4 feature chunks
SCALE = 1.0 / float(np.sqrt(E))

_NC_CACHE = {}


def build_nc(nb=NB, variant=13, reps=1):
    """Build (once) the single-core Bass module processing nb batches.

    variant 1: all six projection layouts via fp32r matmuls from xT.
    variant 2: like 1, but Vt comes from a bf16 DMA(XBAR)-transpose of V
               instead of its own matmul projection (-64 matmuls/batch).
    variant 3: like 2, and Qt/Kt also come from DMA-transposes of the bf16
               natural q/k (row-attention S then runs in bf16;
               -128 more matmuls/batch).
    variant 4: like 3, plus V/Acol/AcolT double-buffered across batches so
               batch b+1's projection phase (PE) can stream while batch b's
               row attention still reads V.
    variant 5: variant 2 + early transposes: exp quarters are transposed
               unnormalized as soon as they exist, and the 1/rowsum lands on
               the PV output (per-partition ACT scale) instead of on P~;
               PV and the col output use separate PSUM tiles.
    variant 6: variant 5 + the double-buffering of variant 4.
    variant 7: variant 6 + 3-deep prow pool.
    """
    FEAT = {
        1: set(),
        2: {"vt_dma"},
        3: {"vt_dma", "qkt_dma"},
        4: {"vt_dma", "qkt_dma", "dbuf"},
        5: {"vt_dma", "early_t"},
        6: {"vt_dma", "early_t", "dbuf"},
        7: {"vt_dma", "early_t", "dbuf", "prow3"},
        8: {"vt_dma", "spread"},
        9: {"vt_dma", "early_t", "spread"},
        10: {"vt_dma", "late_norm", "act_drain"},
        11: {"vt_dma", "act_drain"},
        13: {"vt_dma", "late_norm", "pe_pt", "dve_merge"},
        14: {"vt_dma", "late_norm", "dve_merge"},
        # timing-only diagnostics (wrong results): fake / absent pt transposes
        15: {"vt_dma", "fake_t"},
        16: {"vt_dma", "no_t"},
        17: {"late_norm", "pe_pt", "dve_merge"},
        18: {"vt_dma", "late_norm", "pe_pt", "dve_merge", "prow3"},
        20: {"vt_dma", "late_norm", "pe_pt", "dve_merge", "psum2"},
        22: {"late_norm", "pe_pt", "dve_merge", "f32r_pv", "slice_xt"},
        23: {"late_norm", "pe_pt", "dve_merge", "f32r_pv", "slice_xt", "prow3"},
    }
    if (nb, variant, reps) in _NC_CACHE:
        return _NC_CACHE[(nb, variant, reps)]
    if variant >= 24:
        nc = build_nc_v24(nb, variant, reps)
        _NC_CACHE[(nb, variant, reps)] = nc
        return nc
    feat = FEAT[variant]

    import concourse.bass as bass
    import concourse.tile as tile
    from concourse import bacc, mybir
    from concourse.masks import make_identity

    f32 = mybir.dt.float32
    f32r = mybir.dt.float32r
    bf16 = mybir.dt.bfloat16
    AF = mybir.ActivationFunctionType
    AX = mybir.AxisListType

    nc = bacc.Bacc("TRN2")
    x = nc.dram_tensor("x", [nb, N, E], f32, kind="ExternalInput")
    w = nc.dram_tensor("w", [E, 3 * E], f32, kind="ExternalInput")
    bvec = nc.dram_tensor("b", [3 * E], f32, kind="ExternalInput")
    y = nc.dram_tensor("y", [nb, N, E], f32, kind="ExternalOutput")

    with tile.TileContext(nc) as tc:
        with (
            tc.tile_pool(name="const", bufs=1) as constp,
            tc.tile_pool(name="xn", bufs=2 if ("prow3" in feat or "slice_xt" in feat) else 3) as xnp,
            tc.tile_pool(name="big", bufs=1) as bigp,
            tc.tile_pool(name="xsl", bufs=2) as xslp,
            tc.tile_pool(name="qkn", bufs=3) as qknp,
            tc.tile_pool(name="prow", bufs=3 if "prow3" in feat else 2) as prowp,
            tc.tile_pool(name="stat", bufs=3) as statp,
            tc.tile_pool(name="outp", bufs=2) as outpp,
            tc.tile_pool(name="ps_proj", bufs=2, space="PSUM") as ps_proj,
            tc.tile_pool(name="ps_sc", bufs=4, space="PSUM") as ps_sc,
            tc.tile_pool(name="ps_s", bufs=2, space="PSUM") as ps_s,
        ):
            # ---------------- constants ----------------
            # W lands as float32r (rounded by the DVE copy) so fp32r matmuls
            # accept it; staged through the small xn pool to save SBUF.
            W = constp.tile([128, ED, 3 * E], f32r)
            wv = w[:].rearrange("(k p) m -> p k m", p=128)
            for k in range(ED):
                for c in range(3):
                    wst = xnp.tile([128, E], f32, tag="xn", name=f"wst{k}_{c}")
                    nc.sync.dma_start(wst, wv[:, k, c * E : (c + 1) * E])
                    nc.vector.tensor_copy(W[:, k, c * E : (c + 1) * E], wst)

            # bias broadcast across partitions (for [n-part, e] layouts)
            b3 = bvec[:].rearrange("(c m) -> c m", m=E)
            bb = constp.tile([128, 3, E], bf16)
            nc.gpsimd.dma_start(
                bb, bass.AP(tensor=b3.tensor, offset=b3.offset, ap=[[0, 128]] + list(b3.ap))
            )
            # bias per partition (for [e-part, n] layouts): bpart[p, c] = b[c*128+p]
            bpart = constp.tile([128, 3 * ED], f32)
            nc.gpsimd.dma_start(bpart, bvec[:].rearrange("(c p) -> p c", p=128))

            ident = constp.tile([128, 128], f32)
            make_identity(nc, ident)
            identB = constp.tile([128, 128], bf16)
            make_identity(nc, identB)

            def batch_body():
              for b in range(nb):
                qkt_dt = bf16 if "qkt_dma" in feat else f32r
                vn_dt = f32r if "f32r_pv" in feat else bf16
                dbufs = 2 if "dbuf" in feat else 1
                slice_xt = "slice_xt" in feat
                if not slice_xt:
                    xT = bigp.tile([128, ED, N], f32r, tag="xT")
                Qt = bigp.tile([128, ED, N], qkt_dt, tag="Qt")
                Kt = bigp.tile([128, ED, N], qkt_dt, tag="Kt")
                Vn = bigp.tile([128, NE, E], vn_dt, tag="Vn", bufs=dbufs)
                Vt = bigp.tile([128, ED, N], bf16, tag="Vt")
                Acol = bigp.tile([128, ED, E], bf16, tag="Acol", bufs=dbufs)
                AcolT = bigp.tile([128, ED, E], bf16, tag="AcolT", bufs=dbufs)
                scol_ps = [
                    ps_sc.tile([128, E], f32, tag="scps", name=f"scol{b}_{i}")
                    for i in range(ED)
                ]

                # ---- phase A: load x, build xT, projections, col-S accumulation
                for j in range(NE):
                    s_idx, jj = j // ED, j % ED
                    if slice_xt:
                        if jj == 0:
                            xT = xslp.tile(
                                [128, ED, ED * 128], f32r, tag="xsl", name=f"xsl{b}_{s_idx}"
                            )
                        xoff, roff = jj * 128, 0
                    else:
                        xoff, roff = j * 128, s_idx * 512
                    xn = xnp.tile([128, E], f32, tag="xn")
                    nc.sync.dma_start(xn, x[b, j * 128 : (j + 1) * 128, :])
                    for k in range(ED):
                        tps = ps_proj.tile([128, 128], f32, tag="ps")
                        nc.tensor.transpose(tps, xn[:, k * 128 : (k + 1) * 128], ident)
                        nc.vector.tensor_copy(xT[:, k, xoff : xoff + 128], tps)

                    # natural-layout q, k, v for this token chunk
                    qn = qknp.tile([128, E], bf16, tag="qn")
                    kn = qknp.tile([128, E], bf16, tag="kn")
                    for dst, ci in ((qn, 0), (kn, 1), (Vn[:, j, :], 2)):
                        pp = ps_proj.tile([128, E], f32, tag="ps")
                        for k in range(ED):
                            nc.tensor.matmul(
                                pp,
                                xT[:, k, xoff : xoff + 128],
                                W[:, k, ci * E : (ci + 1) * E],
                                start=(k == 0),
                                stop=(k == ED - 1),
                            )
                        nc.vector.tensor_add(dst, pp, bb[:, ci, :])

                    # col-attention S accumulation: S_col[d,e] += q_j.T @ k_j
                    for i in range(ED):
                        nc.tensor.matmul(
                            scol_ps[i],
                            qn[:, i * 128 : (i + 1) * 128],
                            kn,
                            start=(j == 0),
                            stop=(j == NE - 1),
                        )

                    # bf16 transposed layouts via the DMA XBAR (free wrt PE)
                    jsl = slice(j * 128, (j + 1) * 128)
                    if "vt_dma" in feat:
                        nc.scalar.dma_start_transpose(Vt[:, :, jsl], Vn[:, j, :])
                    if "qkt_dma" in feat:
                        nc.scalar.dma_start_transpose(Qt[:, :, jsl], qn)
                        nc.scalar.dma_start_transpose(Kt[:, :, jsl], kn)

                    # transposed-layout projections, one 512-token slice at a time
                    if "qkt_dma" in feat:
                        tproj = ()
                    elif "vt_dma" in feat:
                        tproj = ((Qt, 0), (Kt, 1))
                    else:
                        tproj = ((Qt, 0), (Kt, 1), (Vt, 2))
                    if j % ED == ED - 1 and tproj:
                        sl = slice(s_idx * 512, (s_idx + 1) * 512)
                        for dst, ci in tproj:
                            for i in range(ED):
                                pp = ps_proj.tile([128, E], f32, tag="ps")
                                for k in range(ED):
                                    nc.tensor.matmul(
                                        pp,
                                        W[:, k, ci * E + i * 128 : ci * E + (i + 1) * 128],
                                        xT[:, k, roff : roff + 512],
                                        start=(k == 0),
                                        stop=(k == ED - 1),
                                    )
                                if "act_drain" in feat:
                                    nc.scalar.activation(
                                        out=dst[:, i, sl],
                                        in_=pp,
                                        func=AF.Identity,
                                        bias=bpart[:, ci * ED + i : ci * ED + i + 1],
                                    )
                                else:
                                    nc.vector.tensor_scalar_add(
                                        dst[:, i, sl], pp, bpart[:, ci * ED + i : ci * ED + i + 1]
                                    )

                # ---- phase A2: col softmax + transpose of A
                # col logits are O(+-600): subtract the per-row max (as an ACT
                # bias of -max*SCALE) before exp, unlike the row path.
                cstat = statp.tile([128, 3 * ED], f32, tag="cstat")
                for i in range(ED):
                    nm = cstat[:, 2 * ED + i : 2 * ED + i + 1]
                    nc.vector.reduce_max(nm, scol_ps[i], axis=AX.X, negate=True)
                    nc.vector.tensor_scalar_mul(nm, nm, SCALE)
                    nc.scalar.activation(
                        out=Acol[:, i, :],
                        in_=scol_ps[i],
                        func=AF.Exp,
                        scale=SCALE,
                        bias=nm,
                        accum_out=cstat[:, i : i + 1],
                    )
                nc.vector.reciprocal(cstat[:, ED : 2 * ED], cstat[:, 0:ED])
                for i in range(ED):
                    nc.vector.tensor_scalar_mul(
                        Acol[:, i, :], Acol[:, i, :], cstat[:, ED + i : ED + i + 1]
                    )
                    nc.scalar.dma_start_transpose(
                        AcolT[:, :, i * 128 : (i + 1) * 128], Acol[:, i, :]
                    )

                # ---- phase B: row attention + merged output, per token chunk
                early_t = "early_t" in feat
                late_norm = "late_norm" in feat
                spread = "spread" in feat
                for j in range(NE):
                    teng = (nc.sync if j % 2 else nc.scalar) if spread else nc.scalar
                    yeng = (nc.scalar if j % 2 else nc.sync) if spread else nc.sync
                    pt = prowp.tile([128, N], bf16, tag="pt")
                    ptT = prowp.tile(
                        [128, NE, 128], f32r if "f32r_pv" in feat else bf16, tag="ptT"
                    )
                    rstat = statp.tile([128, 8], f32, tag="rstat")
                    for q in range(4):
                        sps = ps_s.tile([128, 512], f32, tag="s")
                        for k in range(ED):
                            nc.tensor.matmul(
                                sps,
                                Qt[:, k, j * 128 : (j + 1) * 128],
                                Kt[:, k, q * 512 : (q + 1) * 512],
                                start=(k == 0),
                                stop=(k == ED - 1),
                            )
                        nc.scalar.activation(
                            out=pt[:, q * 512 : (q + 1) * 512],
                            in_=sps,
                            func=AF.Exp,
                            scale=SCALE,
                            accum_out=rstat[:, q : q + 1],
                        )
                        if early_t:
                            # transpose the unnormalized quarter right away;
                            # 1/rowsum is applied to the PV output instead
                            teng.dma_start_transpose(
                                ptT[:, 4 * q : 4 * q + 4, :],
                                pt[:, q * 512 : (q + 1) * 512],
                            )
                        if "pe_pt" in feat:
                            for t in range(4):
                                m = 4 * q + t
                                psB = ps_proj.tile(
                                    [128, 128], bf16, tag="ps", name=f"psB{b}_{j}_{m}"
                                )
                                nc.tensor.transpose(
                                    psB, pt[:, m * 128 : (m + 1) * 128], identB
                                )
                                nc.vector.tensor_copy(ptT[:, m, :], psB)
                    nc.vector.reduce_sum(rstat[:, 4:5], rstat[:, 0:4], axis=AX.X)
                    nc.vector.reciprocal(rstat[:, 5:6], rstat[:, 4:5])
                    if late_norm:
                        if "pe_pt" not in feat:
                            teng.dma_start_transpose(ptT, pt)
                    elif not early_t:
                        nc.vector.tensor_scalar_mul(pt, pt, rstat[:, 5:6])
                        if "fake_t" in feat:
                            teng.dma_start(ptT.rearrange("p a b -> p (a b)"), pt)
                        elif "no_t" in feat:
                            nc.vector.tensor_copy(ptT[:, 0, :], pt[:, :128])
                        else:
                            teng.dma_start_transpose(ptT, pt)

                    po = ps_sc.tile([128, E], f32, tag="scps")
                    for m in range(NE):
                        nc.tensor.matmul(
                            po,
                            ptT[:, m, :],
                            Vn[:, m, :],
                            start=(m == 0),
                            stop=((early_t or late_norm) and m == NE - 1),
                        )
                    ot = outpp.tile([128, E], f32, tag="ot")
                    if early_t or late_norm:
                        oc = ps_sc.tile([128, E], f32, tag="scps")
                        for c in range(ED):
                            nc.tensor.matmul(
                                oc,
                                Vt[:, c, j * 128 : (j + 1) * 128],
                                AcolT[:, c, :],
                                start=(c == 0),
                                stop=(c == ED - 1),
                            )
                        if "dve_merge" in feat:
                            if "psum2" in feat:
                                nc.vector.scalar_tensor_tensor(
                                    ot,
                                    po,
                                    rstat[:, 5:6],
                                    oc,
                                    op0=mybir.AluOpType.mult,
                                    op1=mybir.AluOpType.add,
                                )
                            else:
                                octmp = outpp.tile([128, E], f32, tag="octmp")
                                nc.vector.tensor_copy(octmp, oc)
                                nc.vector.scalar_tensor_tensor(
                                    ot,
                                    po,
                                    rstat[:, 5:6],
                                    octmp,
                                    op0=mybir.AluOpType.mult,
                                    op1=mybir.AluOpType.add,
                                )
                        else:
                            nc.scalar.activation(
                                out=ot, in_=po, func=AF.Copy, scale=rstat[:, 5:6]
                            )
                            nc.vector.tensor_add(ot, ot, oc)
                    else:
                        for c in range(ED):
                            nc.tensor.matmul(
                                po,
                                Vt[:, c, j * 128 : (j + 1) * 128],
                                AcolT[:, c, :],
                                start=False,
                                stop=(c == ED - 1),
                            )
                        nc.vector.tensor_copy(ot, po)
                    yeng.dma_start(y[b, j * 128 : (j + 1) * 128, :], ot)

            if reps == 1:
                batch_body()
            else:
                with tc.For_i(0, reps, 1):
                    batch_body()

    nc.compile()
    _NC_CACHE[(nb, variant, reps)] = nc
    return nc


def make_in_maps(x, w_qkv, b_qkv):
    xs = np.ascontiguousarray(np.asarray(x, dtype=np.float32)).reshape(B, N, E)
    w = np.ascontiguousarray(np.asarray(w_qkv, dtype=np.float32))
    bq = np.ascontiguousarray(np.asarray(b_qkv, dtype=np.float32))
    return [
        {"x": np.ascontiguousarray(xs[c * NB : (c + 1) * NB]), "w": w, "b": bq}
        for c in range(NCORES)
    ]


def build_nc_v24(nb, variant, reps):
    """St-direct + fp8 DoubleRow redesign.

    Key differences vs v13:
      - Row attention computes S TRANSPOSED (St[m-part, j-free]) directly:
        stat=Kt chunk, mov=Qt slice.  exp(St) goes straight to the PV
        stationary layout, eliminating all 256/batch PE transposes of P and
        their 256 DVE PSUM drains.  Row sums come from tiny ones-vector
        matmuls accumulating [128,1] PSUM; 1/rowsum lands on the PV output
        via the DVE merge (late_norm).
      - Row-path matmuls (St, PV, colS) run in fp8e4 with DoubleRow perf
        mode: contraction pairs two 128-partition chunks per instruction.
        Row logits are in [-1.3, 1.0] so exp(s) in [0.27, 2.6] is perfectly
        fp8-representable unnormalized (measured on the reference).
      - Col path (max-sub softmax, colout matmuls) stays bf16: col logits
        are O(+-200) and max-dominated, so fp8 there is risky.
      - Projections stay f32r (fp8 W error is correlated across tokens and
        amplifies through the 2048-term col-logit sums).
      - PSUM drains spread across DVE / ACT / Pool to keep all engines
        under the PE time.

    variants: 24 = base; 25 = 24 + fp8 colout (Vt/AcolT fp8);
    26 = 24 but col-S operands bf16 (no DoubleRow there) — fp8 col logits
    cost 1.4e-2 rel err (max-dominated softmax amplifies logit noise,
    measured), while fp8 anywhere on the row path costs < 7e-4.
    27 = 26 + Qt/Kt from PE transposes of the bf16 qn/kn (Pool fp8 drains)
    instead of their own f32r projection passes (-20.5us/batch PE).
    28 = 27 + col-S as a burst in phase A2 reading Qn/Kn saved in SBUF,
    instead of accumulating in 4 PSUM banks pinned across all of phase A.
    PSUM pools re-split (proj 3 / colS+out 3 / St 2) so phase A of batch
    b+1 and phase B of batch b touch disjoint pools and can overlap.
    Vt transpose issues move ACT -> SP.

    Projections must stay f32r: bf16 x/W gives 2.5e-2 rel err (fails the
    2e-2 gate) because the 2048-term col-logit sums amplify correlated
    weight quantization error; fp8 qk-projection gives 7e-2 (measured).

    29 = 28 with all PSUM drains on DVE/ACT (GPSIMD cannot access PSUM on
    real HW - neuronx-cc birverifier rejects it; CoreSim doesn't model
    that).  The 4 PE transposes of each xT / Qt / Kt chunk group land in
    one PSUM bank (disjoint 128-col slices, skip_group_check) and drain
    with a single wide DVE copy.  Pool keeps only SBUF->SBUF work
    (Vn8 cast, Acol normalize).
    31 = 29 + Qt/Kt merged into one QKt tile so all 8 q/k transposes of a
    token chunk share one PSUM bank and ONE DVE drain; oc PSUM copy moves
    to ACT.
    """
    import concourse.bass as bass
    import concourse.tile as tile
    from concourse import bacc, mybir
    from concourse.masks import make_identity

    f32 = mybir.dt.float32
    f32r = mybir.dt.float32r
    f8 = mybir.dt.float8e4
    # 34 = fp16 replaces f32r for W/xT (same 1 cyc/col PE rate as bf16 vs
    # f32r's ~274 ns/matmul stationary-reload tax; error measured identical
    # to f32r through both paths) and fp16 replaces bf16 everywhere 16-bit.
    if variant >= 34:
        f32r = mybir.dt.float16
        bf16 = mybir.dt.float16
    else:
        bf16 = mybir.dt.bfloat16
    AF = mybir.ActivationFunctionType
    AX = mybir.AxisListType
    DR = mybir.MatmulPerfMode.DoubleRow

    fp8col = variant == 25
    colS_dt = bf16 if variant >= 26 else f8
    qkt_via_transpose = variant >= 27
    colS_burst = variant >= 28
    # no PSUM access from Pool on real HW; 1 = batched bank drains,
    # 2 = alternating single DVE/ACT drains (fallback if HW disagrees with
    # the sim about reading a bank slice re-marked pending-zero)
    hw_legal = 0 if variant < 29 else (2 if variant == 30 else 1)
    precast = variant >= 35  # cast x to fp16 on Pool; 1 cyc/row transposes
    qk_merged = variant >= 31
    resplit = variant == 32  # transposes get their own PSUM pool
    slice_xt = variant >= 33  # xT as rolling 512-token slices + dbuf big tiles

    nc = bacc.Bacc("TRN2")
    x = nc.dram_tensor("x", [nb, N, E], f32, kind="ExternalInput")
    w = nc.dram_tensor("w", [E, 3 * E], f32, kind="ExternalInput")
    bvec = nc.dram_tensor("b", [3 * E], f32, kind="ExternalInput")
    y = nc.dram_tensor("y", [nb, N, E], f32, kind="ExternalOutput")

    with tile.TileContext(nc) as tc:
        with (
            tc.tile_pool(name="const", bufs=1) as constp,
            tc.tile_pool(name="xn", bufs=3) as xnp,
            tc.tile_pool(name="big", bufs=1) as bigp,
            tc.tile_pool(name="qkp", bufs=2) as qkpp,
            tc.tile_pool(name="step", bufs=2) as stepp,
            tc.tile_pool(name="stat", bufs=3) as statp,
            tc.tile_pool(name="outp", bufs=3) as outpp,
            tc.tile_pool(name="ps_proj", bufs=2 if resplit else (3 if colS_burst else 2), space="PSUM") as ps_proj,
            tc.tile_pool(name="ps_sc", bufs=2 if resplit else (3 if colS_burst else 4), space="PSUM") as ps_sc,
            tc.tile_pool(name="ps_s", bufs=2, space="PSUM") as ps_s,
            tc.tile_pool(name="ps_tps", bufs=2, space="PSUM") as ps_tps,
        ):
            # ---------------- constants ----------------
            W = constp.tile([128, ED, 3 * E], f32r)
            wv = w[:].rearrange("(k p) m -> p k m", p=128)
            for k in range(ED):
                for c in range(3):
                    wst = xnp.tile([128, E], f32, tag="xn", name=f"wst{k}_{c}")
                    nc.sync.dma_start(wst, wv[:, k, c * E : (c + 1) * E])
                    nc.vector.tensor_copy(W[:, k, c * E : (c + 1) * E], wst)

            b3 = bvec[:].rearrange("(c m) -> c m", m=E)
            bb = constp.tile([128, 3, E], bf16)
            nc.gpsimd.dma_start(
                bb, bass.AP(tensor=b3.tensor, offset=b3.offset, ap=[[0, 128]] + list(b3.ap))
            )
            bpart = constp.tile([128, 3 * ED], f32)
            nc.gpsimd.dma_start(bpart, bvec[:].rearrange("(c p) -> p c", p=128))

            ident = constp.tile([128, 128], f32)
            make_identity(nc, ident)
            ones8 = constp.tile([128, 2, 1], f8)
            nc.gpsimd.memset(ones8, 1.0)
            if qkt_via_transpose:
                identB = constp.tile([128, 128], bf16)
                make_identity(nc, identB)

            def batch_body():
              for b in range(nb):
                if not slice_xt:
                    xT = bigp.tile([128, ED, N], f32r, tag="xT")
                if qk_merged:
                    QKt = bigp.tile([128, 2, ED, N], f8, tag="QKt",
                                    bufs=2 if slice_xt else 1)
                    Qt, Kt = QKt[:, 0], QKt[:, 1]
                else:
                    Qt = bigp.tile([128, ED, N], f8, tag="Qt")
                    Kt = bigp.tile([128, ED, N], f8, tag="Kt")
                Vn8 = bigp.tile([128, NE, E], f8, tag="Vn8")
                Vnb = bigp.tile([128, NE, E], bf16, tag="Vnb")
                vt_dt = f8 if fp8col else bf16
                Vt = bigp.tile([128, ED, N], vt_dt, tag="Vt")
                Acol = bigp.tile([128, ED, E], bf16, tag="Acol",
                                 bufs=2 if slice_xt else 1)
                AcolT = bigp.tile([128, ED, E], vt_dt, tag="AcolT",
                                  bufs=2 if slice_xt else 1)
                if colS_burst:
                    Qn = bigp.tile([128, NE, E], bf16, tag="Qn")
                    Kn = bigp.tile([128, NE, E], bf16, tag="Kn")
                    scol_ps = None
                else:
                    scol_ps = [
                        ps_sc.tile([128, E], f32, tag="scps", name=f"scol{b}_{i}")
                        for i in range(ED)
                    ]

                # ---- phase A: load x, build xT, projections, col-S accum
                qp = kp = None
                for j in range(NE):
                    s_idx = j // ED
                    xoff, roff = j * 128, s_idx * 512
                    if slice_xt:
                        if j % ED == 0:
                            xT = qkpp.tile(
                                [128, ED, 512], f32r, tag="xsl", name=f"xsl{b}_{s_idx}"
                            )
                        xoff = (j % ED) * 128
                    xn = xnp.tile([128, E], f32, tag="xn")
                    nc.sync.dma_start(xn, x[b, j * 128 : (j + 1) * 128, :])
                    if precast:
                        xnh = xnp.tile([128, E], bf16, tag="xnh")
                        nc.gpsimd.tensor_copy(xnh, xn)
                        xn = xnh
                    t_dt, t_id = (bf16, identB) if precast else (f32, ident)
                    if hw_legal == 1:
                        tpool = ps_tps if resplit else ps_proj
                        tpsx = tpool.tile([128, ED, 128], t_dt, tag="tps" if resplit else "ps", name=f"tpsx{b}_{j}")
                        for k in range(ED):
                            nc.tensor.matmul(
                                tpsx[:, k, :],
                                xn[:, k * 128 : (k + 1) * 128],
                                t_id,
                                is_transpose=True,
                                skip_group_check=True,
                            )
                        nc.vector.tensor_copy(xT[:, :, xoff : xoff + 128], tpsx)
                    elif hw_legal:
                        for k in range(ED):
                            tps = ps_proj.tile([128, 128], f32, tag="ps")
                            nc.tensor.transpose(tps, xn[:, k * 128 : (k + 1) * 128], ident)
                            eng = nc.vector if k % 2 == 0 else nc.scalar
                            if k % 2 == 0:
                                eng.tensor_copy(xT[:, k, xoff : xoff + 128], tps)
                            else:
                                eng.copy(xT[:, k, xoff : xoff + 128], tps)
                    else:
                        for k in range(ED):
                            tps = ps_proj.tile([128, 128], f32, tag="ps")
                            nc.tensor.transpose(tps, xn[:, k * 128 : (k + 1) * 128], ident)
                            nc.gpsimd.tensor_copy(xT[:, k, xoff : xoff + 128], tps)

                    # natural-layout q, k (pair-staged or SBUF-resident), v
                    if colS_burst:
                        qdst, kdst = Qn[:, j, :], Kn[:, j, :]
                    else:
                        if j % 2 == 0:
                            qp = qkpp.tile([128, 2, E], colS_dt, tag="qp", name=f"qp{b}_{j}")
                            kp = qkpp.tile([128, 2, E], colS_dt, tag="kp", name=f"kp{b}_{j}")
                        qdst, kdst = qp[:, j % 2, :], kp[:, j % 2, :]
                    for dst, ci in (
                        (qdst, 0),
                        (kdst, 1),
                        (Vnb[:, j, :], 2),
                    ):
                        pp = ps_proj.tile([128, E], f32, tag="ps")
                        for k in range(ED):
                            nc.tensor.matmul(
                                pp,
                                xT[:, k, xoff : xoff + 128],
                                W[:, k, ci * E : (ci + 1) * E],
                                start=(k == 0),
                                stop=(k == ED - 1),
                            )
                        nc.vector.tensor_add(dst, pp, bb[:, ci, :])
                    nc.gpsimd.tensor_copy(Vn8[:, j, :], Vnb[:, j, :])
                    if not fp8col:
                        vt_eng = nc.sync if colS_burst else nc.scalar
                        vt_eng.dma_start_transpose(
                            Vt[:, :, j * 128 : (j + 1) * 128], Vnb[:, j, :]
                        )

                    # col-S accumulation, every second chunk
                    if not colS_burst and j % 2 == 1:
                        if colS_dt is f8:
                            for i in range(ED):
                                nc.tensor.matmul(
                                    scol_ps[i],
                                    qp[:, :, i * 128 : (i + 1) * 128],
                                    kp,
                                    start=(j == 1),
                                    stop=(j == NE - 1),
                                    perf_mode=DR,
                                )
                        else:
                            for jj in range(2):
                                for i in range(ED):
                                    nc.tensor.matmul(
                                        scol_ps[i],
                                        qp[:, jj, i * 128 : (i + 1) * 128],
                                        kp[:, jj, :],
                                        start=(j == 1 and jj == 0),
                                        stop=(j == NE - 1 and jj == 1),
                                    )

                    # Qt/Kt transposed layouts
                    if qkt_via_transpose and qk_merged:
                        qkpool = ps_tps if resplit else ps_proj
                        psqk = qkpool.tile(
                            [128, 2, ED, 128], bf16, tag="tps" if resplit else "ps", name=f"psqk{b}_{j}"
                        )
                        for ci, src in ((0, qdst), (1, kdst)):
                            for i in range(ED):
                                nc.tensor.matmul(
                                    psqk[:, ci, i, :],
                                    src[:, i * 128 : (i + 1) * 128],
                                    identB,
                                    is_transpose=True,
                                    skip_group_check=True,
                                )
                        nc.vector.tensor_copy(QKt[:, :, :, j * 128 : (j + 1) * 128], psqk)
                    elif qkt_via_transpose:
                        for (dst, src, ci) in ((Qt, qdst, 0), (Kt, kdst, 1)):
                            if hw_legal == 1:
                                psB = ps_proj.tile(
                                    [128, ED, 128], bf16, tag="ps", name=f"psB{b}_{j}_{ci}"
                                )
                                for i in range(ED):
                                    nc.tensor.matmul(
                                        psB[:, i, :],
                                        src[:, i * 128 : (i + 1) * 128],
                                        identB,
                                        is_transpose=True,
                                        skip_group_check=True,
                                    )
                                nc.vector.tensor_copy(
                                    dst[:, :, j * 128 : (j + 1) * 128], psB
                                )
                            else:
                                for i in range(ED):
                                    psB = ps_proj.tile(
                                        [128, 128], bf16, tag="ps", name=f"psB{b}_{j}_{ci}_{i}"
                                    )
                                    nc.tensor.transpose(
                                        psB, src[:, i * 128 : (i + 1) * 128], identB
                                    )
                                    if hw_legal:
                                        if i % 2 == 0:
                                            nc.vector.tensor_copy(
                                                dst[:, i, j * 128 : (j + 1) * 128], psB
                                            )
                                        else:
                                            nc.scalar.copy(
                                                dst[:, i, j * 128 : (j + 1) * 128], psB
                                            )
                                    else:
                                        nc.gpsimd.tensor_copy(
                                            dst[:, i, j * 128 : (j + 1) * 128], psB
                                        )
                    elif j % ED == ED - 1:
                        sl = slice(s_idx * 512, (s_idx + 1) * 512)
                        for dst, ci in ((Qt, 0), (Kt, 1)):
                            for i in range(ED):
                                pp = ps_proj.tile([128, E], f32, tag="ps")
                                for k in range(ED):
                                    nc.tensor.matmul(
                                        pp,
                                        W[:, k, ci * E + i * 128 : ci * E + (i + 1) * 128],
                                        xT[:, k, roff : roff + 512],
                                        start=(k == 0),
                                        stop=(k == ED - 1),
                                    )
                                nc.scalar.activation(
                                    out=dst[:, i, sl],
                                    in_=pp,
                                    func=AF.Identity,
                                    bias=bpart[:, ci * ED + i : ci * ED + i + 1],
                                )

                # ---- phase A2: col softmax (max-sub) + transpose of A
                cstat = statp.tile([128, 3 * ED], f32, tag="cstat")
                for i in range(ED):
                    if colS_burst:
                        scps = ps_sc.tile([128, E], f32, tag="scps", name=f"scol{b}_{i}")
                        for jc in range(NE):
                            nc.tensor.matmul(
                                scps,
                                Qn[:, jc, i * 128 : (i + 1) * 128],
                                Kn[:, jc, :],
                                start=(jc == 0),
                                stop=(jc == NE - 1),
                            )
                    else:
                        scps = scol_ps[i]
                    nm = cstat[:, 2 * ED + i : 2 * ED + i + 1]
                    nc.vector.reduce_max(nm, scps, axis=AX.X, negate=True)
                    nc.vector.tensor_scalar_mul(nm, nm, SCALE)
                    nc.scalar.activation(
                        out=Acol[:, i, :],
                        in_=scps,
                        func=AF.Exp,
                        scale=SCALE,
                        bias=nm,
                        accum_out=cstat[:, i : i + 1],
                    )
                nc.vector.reciprocal(cstat[:, ED : 2 * ED], cstat[:, 0:ED])
                norm_eng = nc.gpsimd if hw_legal else nc.vector
                for i in range(ED):
                    norm_eng.tensor_scalar_mul(
                        Acol[:, i, :], Acol[:, i, :], cstat[:, ED + i : ED + i + 1]
                    )
                    if not fp8col:
                        nc.scalar.dma_start_transpose(
                            AcolT[:, :, i * 128 : (i + 1) * 128], Acol[:, i, :]
                        )

                # ---- phase B: St-direct row attention + merged output
                for s in range(ED):
                    ssl = slice(s * 512, (s + 1) * 512)
                    StE = stepp.tile([128, NE, 512], f8, tag="ste", name=f"ste{b}_{s}")
                    for m in range(NE):
                        sps = ps_s.tile([128, 512], f32, tag="s")
                        for kk in range(2):
                            nc.tensor.matmul(
                                sps,
                                Kt[:, 2 * kk : 2 * kk + 2, m * 128 : (m + 1) * 128],
                                Qt[:, 2 * kk : 2 * kk + 2, ssl],
                                start=(kk == 0),
                                stop=(kk == 1),
                                perf_mode=DR,
                            )
                        nc.scalar.activation(
                            out=StE[:, m, :], in_=sps, func=AF.Exp, scale=SCALE
                        )
                    rs_ps = ps_sc.tile([128, 4], f32, tag="scps", name=f"rs{b}_{s}")
                    for jj in range(4):
                        for mm in range(8):
                            nc.tensor.matmul(
                                rs_ps[:, jj : jj + 1],
                                StE[:, 2 * mm : 2 * mm + 2, jj * 128 : (jj + 1) * 128],
                                ones8,
                                start=(mm == 0),
                                stop=(mm == 7),
                                perf_mode=DR,
                                skip_group_check=True,
                            )
                    rstat = statp.tile([128, 4], f32, tag="rstat")
                    nc.vector.reciprocal(rstat, rs_ps)
                    for jj in range(4):
                        j = s * 4 + jj
                        jsl = slice(j * 128, (j + 1) * 128)
                        po = ps_sc.tile([128, E], f32, tag="scps")
                        for mm in range(8):
                            nc.tensor.matmul(
                                po,
                                StE[:, 2 * mm : 2 * mm + 2, jj * 128 : (jj + 1) * 128],
                                Vn8[:, 2 * mm : 2 * mm + 2, :],
                                start=(mm == 0),
                                stop=(mm == 7),
                                perf_mode=DR,
                            )
                        oc = ps_sc.tile([128, E], f32, tag="scps")
                        if fp8col:
                            for c in range(2):
                                nc.tensor.matmul(
                                    oc,
                                    Vt[:, 2 * c : 2 * c + 2, jsl],
                                    AcolT[:, 2 * c : 2 * c + 2, :],
                                    start=(c == 0),
                                    stop=(c == 1),
                                    perf_mode=DR,
                                )
                        else:
                            for c in range(ED):
                                nc.tensor.matmul(
                                    oc,
                                    Vt[:, c, jsl],
                                    AcolT[:, c, :],
                                    start=(c == 0),
                                    stop=(c == ED - 1),
                                )
                        octmp = outpp.tile([128, E], f32, tag="octmp")
                        if qk_merged:
                            nc.scalar.copy(octmp, oc)
                        elif hw_legal:
                            nc.vector.tensor_copy(octmp, oc)
                        else:
                            nc.gpsimd.tensor_copy(octmp, oc)
                        ot = outpp.tile([128, E], f32, tag="ot")
                        nc.vector.scalar_tensor_tensor(
                            ot,
                            po,
                            rstat[:, jj : jj + 1],
                            octmp,
                            op0=mybir.AluOpType.mult,
                            op1=mybir.AluOpType.add,
                        )
                        nc.sync.dma_start(y[b, jsl, :], ot)

            if reps == 1:
                batch_body()
            else:
                with tc.For_i(0, reps, 1):
                    batch_body()

    nc.compile()
    return nc


BEST_VARIANT = 34


def kernel(x, w_qkv, b_qkv):
    from concourse.bass_utils import run_bass_kernel_spmd

    nc = build_nc(NB, BEST_VARIANT)
    in_maps = make_in_maps(x, w_qkv, b_qkv)
    res = run_bass_kernel_spmd(nc, in_maps, core_ids=list(range(NCORES)))
    out = np.empty((B, N, E), dtype=np.float32)
    for c in range(NCORES):
        out[c * NB : (c + 1) * NB] = res.results[c]["y"]
    return out



# revision 47
# speedup vs baseline: 1.0257x; 1.0257x over previous
"""AxialSelfAttention Trainium2 Bass kernel.

Reference computation (per batch b):
    xs  = x[b] reshaped [N=2048, E=512]
    qkv = xs @ W + bias                      # [N, 3E]
    q, k, v = split(qkv)
    row:  P = softmax(q @ k.T / sqrt(E));  out_row = P @ v
    col:  A = softmax(q.T @ k / sqrt(E));  out_col = v @ A.T
    out = out_row + out_col                  # [N, E]

Sharding: data-parallel over batch B=32 across 8 cores (4 batches/core).

Best variant (35, see build_nc_v24): St-direct row attention + fp8
DoubleRow + fp16 projections.  Per batch:
  - x chunks are cast to fp16 on Pool, PE-transposed (1 cyc/row) into
    rolling 512-token xT slices; q/k/v projections are fp16 matmuls from
    xT.  fp16 is the precision sweet spot: bf16 projections fail the gate
    (2.5e-2 - the 2048-term col-logit sums amplify correlated weight
    quantization error 6x) while fp16 is indistinguishable from f32r
    (measured) and avoids f32r's ~80 ns/matmul stationary-reload tax.
  - Natural q/k (fp16) are saved in SBUF; col-S runs as a burst of fp16
    matmuls in rotating PSUM banks, then max-subtracted softmax -> Acol,
    DMA-XBAR-transposed to AcolT.
  - Qt/Kt come from fp16 PE transposes of q/k, drained fp8 into one merged
    QKt tile (all 8 transposes of a chunk share one PSUM bank + one DVE
    drain).
  - Row attention computes S TRANSPOSED (stat=Kt chunk, mov=Qt slice) in
    fp8 DoubleRow; exp(St) lands directly in the PV stationary layout
    (fp8, unnormalized - row logits are in [-1.3, 1.0] so exp fits fp8).
    Row sums come from ones-vector DoubleRow matmuls; 1/rowsum is applied
    in the final DVE merge (out = po * rinv + out_col).
  - PV and colout accumulate in separate PSUM tiles; merged on DVE.
Measured on HW (8 cores, reps-slope, matched-epoch A/B): v13 baseline
1.10 ms -> v33 0.84 ms -> v35 ~0.72 ms per invocation; rel err 1.2e-3.
"""

import sys

for _p in ("/opt/trn_rl_repo", "/root/.axon_site/_ro/trn_rl_repo"):
    if _p not in sys.path:
        sys.path.append(_p)

import numpy as np

B, N, E = 32, 2048, 512
NCORES = 8
NB = B // NCORES  # batches per core
NE = N // 128  # 16 token chunks
ED = E // 128  # 4 feature chunks
SCALE = 1.0 / float(np.sqrt(E))

_NC_CACHE = {}


def build_nc(nb=NB, variant=13, reps=1):
    """Build (once) the single-core Bass module processing nb batches.

    variant 1: all six projection layouts via fp32r matmuls from xT.
    variant 2: like 1, but Vt comes from a bf16 DMA(XBAR)-transpose of V
               instead of its own matmul projection (-64 matmuls/batch).
    variant 3: like 2, and Qt/Kt also come from DMA-transposes of the bf16
               natural q/k (row-attention S then runs in bf16;
               -128 more matmuls/batch).
    variant 4: like 3, plus V/Acol/AcolT double-buffered across batches so
               batch b+1's projection phase (PE) can stream while batch b's
               row attention still reads V.
    variant 5: variant 2 + early transposes: exp quarters are transposed
               unnormalized as soon as they exist, and the 1/rowsum lands on
               the PV output (per-partition ACT scale) instead of on P~;
               PV and the col output use separate PSUM tiles.
    variant 6: variant 5 + the double-buffering of variant 4.
    variant 7: variant 6 + 3-deep prow pool.
    """
    FEAT = {
        1: set(),
        2: {"vt_dma"},
        3: {"vt_dma", "qkt_dma"},
        4: {"vt_dma", "qkt_dma", "dbuf"},
        5: {"vt_dma", "early_t"},
        6: {"vt_dma", "early_t", "dbuf"},
        7: {"vt_dma", "early_t", "dbuf", "prow3"},
        8: {"vt_dma", "spread"},
        9: {"vt_dma", "early_t", "spread"},
        10: {"vt_dma", "late_norm", "act_drain"},
        11: {"vt_dma", "act_drain"},
        13: {"vt_dma", "late_norm", "pe_pt", "dve_merge"},
        14: {"vt_dma", "late_norm", "dve_merge"},
        # timing-only diagnostics (wrong results): fake / absent pt transposes
        15: {"vt_dma", "fake_t"},
        16: {"vt_dma", "no_t"},
        17: {"late_norm", "pe_pt", "dve_merge"},
        18: {"vt_dma", "late_norm", "pe_pt", "dve_merge", "prow3"},
        20: {"vt_dma", "late_norm", "pe_pt", "dve_merge", "psum2"},
        22: {"late_norm", "pe_pt", "dve_merge", "f32r_pv", "slice_xt"},
        23: {"late_norm", "pe_pt", "dve_merge", "f32r_pv", "slice_xt", "prow3"},
    }
    if (nb, variant, reps) in _NC_CACHE:
        return _NC_CACHE[(nb, variant, reps)]
    if variant >= 24:
        nc = build_nc_v24(nb, variant, reps)
        _NC_CACHE[(nb, variant, reps)] = nc
        return nc
    feat = FEAT[variant]

    import concourse.bass as bass
    import concourse.tile as tile
    from concourse import bacc, mybir
    from concourse.masks import make_identity

    f32 = mybir.dt.float32
    f32r = mybir.dt.float32r
    bf16 = mybir.dt.bfloat16
    AF = mybir.ActivationFunctionType
    AX = mybir.AxisListType

    nc = bacc.Bacc("TRN2")
    x = nc.dram_tensor("x", [nb, N, E], f32, kind="ExternalInput")
    w = nc.dram_tensor("w", [E, 3 * E], f32, kind="ExternalInput")
    bvec = nc.dram_tensor("b", [3 * E], f32, kind="ExternalInput")
    y = nc.dram_tensor("y", [nb, N, E], f32, kind="ExternalOutput")

    with tile.TileContext(nc) as tc:
        with (
            tc.tile_pool(name="const", bufs=1) as constp,
            tc.tile_pool(name="xn", bufs=2 if ("prow3" in feat or "slice_xt" in feat) else 3) as xnp,
            tc.tile_pool(name="big", bufs=1) as bigp,
            tc.tile_pool(name="xsl", bufs=2) as xslp,
            tc.tile_pool(name="qkn", bufs=3) as qknp,
            tc.tile_pool(name="prow", bufs=3 if "prow3" in feat else 2) as prowp,
            tc.tile_pool(name="stat", bufs=3) as statp,
            tc.tile_pool(name="outp", bufs=2) as outpp,
            tc.tile_pool(name="ps_proj", bufs=2, space="PSUM") as ps_proj,
            tc.tile_pool(name="ps_sc", bufs=4, space="PSUM") as ps_sc,
            tc.tile_pool(name="ps_s", bufs=2, space="PSUM") as ps_s,
        ):
            # ---------------- constants ----------------
            # W lands as float32r (rounded by the DVE copy) so fp32r matmuls
            # accept it; staged through the small xn pool to save SBUF.
            W = constp.tile([128, ED, 3 * E], f32r)
            wv = w[:].rearrange("(k p) m -> p k m", p=128)
            for k in range(ED):
                for c in range(3):
                    wst = xnp.tile([128, E], f32, tag="xn", name=f"wst{k}_{c}")
                    nc.sync.dma_start(wst, wv[:, k, c * E : (c + 1) * E])
                    nc.vector.tensor_copy(W[:, k, c * E : (c + 1) * E], wst)

            # bias broadcast across partitions (for [n-part, e] layouts)
            b3 = bvec[:].rearrange("(c m) -> c m", m=E)
            bb = constp.tile([128, 3, E], bf16)
            nc.gpsimd.dma_start(
                bb, bass.AP(tensor=b3.tensor, offset=b3.offset, ap=[[0, 128]] + list(b3.ap))
            )
            # bias per partition (for [e-part, n] layouts): bpart[p, c] = b[c*128+p]
            bpart = constp.tile([128, 3 * ED], f32)
            nc.gpsimd.dma_start(bpart, bvec[:].rearrange("(c p) -> p c", p=128))

            ident = constp.tile([128, 128], f32)
            make_identity(nc, ident)
            identB = constp.tile([128, 128], bf16)
            make_identity(nc, identB)

            def batch_body():
              for b in range(nb):
                qkt_dt = bf16 if "qkt_dma" in feat else f32r
                vn_dt = f32r if "f32r_pv" in feat else bf16
                dbufs = 2 if "dbuf" in feat else 1
                slice_xt = "slice_xt" in feat
                if not slice_xt:
                    xT = bigp.tile([128, ED, N], f32r, tag="xT")
                Qt = bigp.tile([128, ED, N], qkt_dt, tag="Qt")
                Kt = bigp.tile([128, ED, N], qkt_dt, tag="Kt")
                Vn = bigp.tile([128, NE, E], vn_dt, tag="Vn", bufs=dbufs)
                Vt = bigp.tile([128, ED, N], bf16, tag="Vt")
                Acol = bigp.tile([128, ED, E], bf16, tag="Acol", bufs=dbufs)
                AcolT = bigp.tile([128, ED, E], bf16, tag="AcolT", bufs=dbufs)
                scol_ps = [
                    ps_sc.tile([128, E], f32, tag="scps", name=f"scol{b}_{i}")
                    for i in range(ED)
                ]

                # ---- phase A: load x, build xT, projections, col-S accumulation
                for j in range(NE):
                    s_idx, jj = j // ED, j % ED
                    if slice_xt:
                        if jj == 0:
                            xT = xslp.tile(
                                [128, ED, ED * 128], f32r, tag="xsl", name=f"xsl{b}_{s_idx}"
                            )
                        xoff, roff = jj * 128, 0
                    else:
                        xoff, roff = j * 128, s_idx * 512
                    xn = xnp.tile([128, E], f32, tag="xn")
                    nc.sync.dma_start(xn, x[b, j * 128 : (j + 1) * 128, :])
                    for k in range(ED):
                        tps = ps_proj.tile([128, 128], f32, tag="ps")
                        nc.tensor.transpose(tps, xn[:, k * 128 : (k + 1) * 128], ident)
                        nc.vector.tensor_copy(xT[:, k, xoff : xoff + 128], tps)

                    # natural-layout q, k, v for this token chunk
                    qn = qknp.tile([128, E], bf16, tag="qn")
                    kn = qknp.tile([128, E], bf16, tag="kn")
                    for dst, ci in ((qn, 0), (kn, 1), (Vn[:, j, :], 2)):
                        pp = ps_proj.tile([128, E], f32, tag="ps")
                        for k in range(ED):
                            nc.tensor.matmul(
                                pp,
                                xT[:, k, xoff : xoff + 128],
                                W[:, k, ci * E : (ci + 1) * E],
                                start=(k == 0),
                                stop=(k == ED - 1),
                            )
                        nc.vector.tensor_add(dst, pp, bb[:, ci, :])

                    # col-attention S accumulation: S_col[d,e] += q_j.T @ k_j
                    for i in range(ED):
                        nc.tensor.matmul(
                            scol_ps[i],
                            qn[:, i * 128 : (i + 1) * 128],
                            kn,
                            start=(j == 0),
                            stop=(j == NE - 1),
                        )

                    # bf16 transposed layouts via the DMA XBAR (free wrt PE)
                    jsl = slice(j * 128, (j + 1) * 128)
                    if "vt_dma" in feat:
                        nc.scalar.dma_start_transpose(Vt[:, :, jsl], Vn[:, j, :])
                    if "qkt_dma" in feat:
                        nc.scalar.dma_start_transpose(Qt[:, :, jsl], qn)
                        nc.scalar.dma_start_transpose(Kt[:, :, jsl], kn)

                    # transposed-layout projections, one 512-token slice at a time
                    if "qkt_dma" in feat:
                        tproj = ()
                    elif "vt_dma" in feat:
                        tproj = ((Qt, 0), (Kt, 1))
                    else:
                        tproj = ((Qt, 0), (Kt, 1), (Vt, 2))
                    if j % ED == ED - 1 and tproj:
                        sl = slice(s_idx * 512, (s_idx + 1) * 512)
                        for dst, ci in tproj:
                            for i in range(ED):
                                pp = ps_proj.tile([128, E], f32, tag="ps")
                                for k in range(ED):
                                    nc.tensor.matmul(
                                        pp,
                                        W[:, k, ci * E + i * 128 : ci * E + (i + 1) * 128],
                                        xT[:, k, roff : roff + 512],
                                        start=(k == 0),
                                        stop=(k == ED - 1),
                                    )
                                if "act_drain" in feat:
                                    nc.scalar.activation(
                                        out=dst[:, i, sl],
                                        in_=pp,
                                        func=AF.Identity,
                                        bias=bpart[:, ci * ED + i : ci * ED + i + 1],
                                    )
                                else:
                                    nc.vector.tensor_scalar_add(
                                        dst[:, i, sl], pp, bpart[:, ci * ED + i : ci * ED + i + 1]
                                    )

                # ---- phase A2: col softmax + transpose of A
                # col logits are O(+-600): subtract the per-row max (as an ACT
                # bias of -max*SCALE) before exp, unlike the row path.
                cstat = statp.tile([128, 3 * ED], f32, tag="cstat")
                for i in range(ED):
                    nm = cstat[:, 2 * ED + i : 2 * ED + i + 1]
                    nc.vector.reduce_max(nm, scol_ps[i], axis=AX.X, negate=True)
                    nc.vector.tensor_scalar_mul(nm, nm, SCALE)
                    nc.scalar.activation(
                        out=Acol[:, i, :],
                        in_=scol_ps[i],
                        func=AF.Exp,
                        scale=SCALE,
                        bias=nm,
                        accum_out=cstat[:, i : i + 1],
                    )
                nc.vector.reciprocal(cstat[:, ED : 2 * ED], cstat[:, 0:ED])
                for i in range(ED):
                    nc.vector.tensor_scalar_mul(
                        Acol[:, i, :], Acol[:, i, :], cstat[:, ED + i : ED + i + 1]
                    )
                    nc.scalar.dma_start_transpose(
                        AcolT[:, :, i * 128 : (i + 1) * 128], Acol[:, i, :]
                    )

                # ---- phase B: row attention + merged output, per token chunk
                early_t = "early_t" in feat
                late_norm = "late_norm" in feat
                spread = "spread" in feat
                for j in range(NE):
                    teng = (nc.sync if j % 2 else nc.scalar) if spread else nc.scalar
                    yeng = (nc.scalar if j % 2 else nc.sync) if spread else nc.sync
                    pt = prowp.tile([128, N], bf16, tag="pt")
                    ptT = prowp.tile(
                        [128, NE, 128], f32r if "f32r_pv" in feat else bf16, tag="ptT"
                    )
                    rstat = statp.tile([128, 8], f32, tag="rstat")
                    for q in range(4):
                        sps = ps_s.tile([128, 512], f32, tag="s")
                        for k in range(ED):
                            nc.tensor.matmul(
                                sps,
                                Qt[:, k, j * 128 : (j + 1) * 128],
                                Kt[:, k, q * 512 : (q + 1) * 512],
                                start=(k == 0),
                                stop=(k == ED - 1),
                            )
                        nc.scalar.activation(
                            out=pt[:, q * 512 : (q + 1) * 512],
                            in_=sps,
                            func=AF.Exp,
                            scale=SCALE,
                            accum_out=rstat[:, q : q + 1],
                        )
                        if early_t:
                            # transpose the unnormalized quarter right away;
                            # 1/rowsum is applied to the PV output instead
                            teng.dma_start_transpose(
                                ptT[:, 4 * q : 4 * q + 4, :],
                                pt[:, q * 512 : (q + 1) * 512],
                            )
                        if "pe_pt" in feat:
                            for t in range(4):
                                m = 4 * q + t
                                psB = ps_proj.tile(
                                    [128, 128], bf16, tag="ps", name=f"psB{b}_{j}_{m}"
                                )
                                nc.tensor.transpose(
                                    psB, pt[:, m * 128 : (m + 1) * 128], identB
                                )
                                nc.vector.tensor_copy(ptT[:, m, :], psB)
                    nc.vector.reduce_sum(rstat[:, 4:5], rstat[:, 0:4], axis=AX.X)
                    nc.vector.reciprocal(rstat[:, 5:6], rstat[:, 4:5])
                    if late_norm:
                        if "pe_pt" not in feat:
                            teng.dma_start_transpose(ptT, pt)
                    elif not early_t:
                        nc.vector.tensor_scalar_mul(pt, pt, rstat[:, 5:6])
                        if "fake_t" in feat:
                            teng.dma_start(ptT.rearrange("p a b -> p (a b)"), pt)
                        elif "no_t" in feat:
                            nc.vector.tensor_copy(ptT[:, 0, :], pt[:, :128])
                        else:
                            teng.dma_start_transpose(ptT, pt)

                    po = ps_sc.tile([128, E], f32, tag="scps")
                    for m in range(NE):
                        nc.tensor.matmul(
                            po,
                            ptT[:, m, :],
                            Vn[:, m, :],
                            start=(m == 0),
                            stop=((early_t or late_norm) and m == NE - 1),
                        )
                    ot = outpp.tile([128, E], f32, tag="ot")
                    if early_t or late_norm:
                        oc = ps_sc.tile([128, E], f32, tag="scps")
                        for c in range(ED):
                            nc.tensor.matmul(
                                oc,
                                Vt[:, c, j * 128 : (j + 1) * 128],
                                AcolT[:, c, :],
                                start=(c == 0),
                                stop=(c == ED - 1),
                            )
                        if "dve_merge" in feat:
                            if "psum2" in feat:
                                nc.vector.scalar_tensor_tensor(
                                    ot,
                                    po,
                                    rstat[:, 5:6],
                                    oc,
                                    op0=mybir.AluOpType.mult,
                                    op1=mybir.AluOpType.add,
                                )
                            else:
                                octmp = outpp.tile([128, E], f32, tag="octmp")
                                nc.vector.tensor_copy(octmp, oc)
                                nc.vector.scalar_tensor_tensor(
                                    ot,
                                    po,
                                    rstat[:, 5:6],
                                    octmp,
                                    op0=mybir.AluOpType.mult,
                                    op1=mybir.AluOpType.add,
                                )
                        else:
                            nc.scalar.activation(
                                out=ot, in_=po, func=AF.Copy, scale=rstat[:, 5:6]
                            )
                            nc.vector.tensor_add(ot, ot, oc)
                    else:
                        for c in range(ED):
                            nc.tensor.matmul(
                                po,
                                Vt[:, c, j * 128 : (j + 1) * 128],
                                AcolT[:, c, :],
                                start=False,
                                stop=(c == ED - 1),
                            )
                        nc.vector.tensor_copy(ot, po)
                    yeng.dma_start(y[b, j * 128 : (j + 1) * 128, :], ot)

            if reps == 1:
                batch_body()
            else:
                with tc.For_i(0, reps, 1):
                    batch_body()

    nc.compile()
    _NC_CACHE[(nb, variant, reps)] = nc
    return nc


def make_in_maps(x, w_qkv, b_qkv):
    xs = np.ascontiguousarray(np.asarray(x, dtype=np.float32)).reshape(B, N, E)
    w = np.ascontiguousarray(np.asarray(w_qkv, dtype=np.float32))
    bq = np.ascontiguousarray(np.asarray(b_qkv, dtype=np.float32))
    return [
        {"x": np.ascontiguousarray(xs[c * NB : (c + 1) * NB]), "w": w, "b": bq}
        for c in range(NCORES)
    ]


def build_nc_v24(nb, variant, reps):
    """St-direct + fp8 DoubleRow redesign.

    Key differences vs v13:
      - Row attention computes S TRANSPOSED (St[m-part, j-free]) directly:
        stat=Kt chunk, mov=Qt slice.  exp(St) goes straight to the PV
        stationary layout, eliminating all 256/batch PE transposes of P and
        their 256 DVE PSUM drains.  Row sums come from tiny ones-vector
        matmuls accumulating [128,1] PSUM; 1/rowsum lands on the PV output
        via the DVE merge (late_norm).
      - Row-path matmuls (St, PV, colS) run in fp8e4 with DoubleRow perf
        mode: contraction pairs two 128-partition chunks per instruction.
        Row logits are in [-1.3, 1.0] so exp(s) in [0.27, 2.6] is perfectly
        fp8-representable unnormalized (measured on the reference).
      - Col path (max-sub softmax, colout matmuls) stays bf16: col logits
        are O(+-200) and max-dominated, so fp8 there is risky.
      - Projections stay f32r (fp8 W error is correlated across tokens and
        amplifies through the 2048-term col-logit sums).
      - PSUM drains spread across DVE / ACT / Pool to keep all engines
        under the PE time.

    variants: 24 = base; 25 = 24 + fp8 colout (Vt/AcolT fp8);
    26 = 24 but col-S operands bf16 (no DoubleRow there) — fp8 col logits
    cost 1.4e-2 rel err (max-dominated softmax amplifies logit noise,
    measured), while fp8 anywhere on the row path costs < 7e-4.
    27 = 26 + Qt/Kt from PE transposes of the bf16 qn/kn (Pool fp8 drains)
    instead of their own f32r projection passes (-20.5us/batch PE).
    28 = 27 + col-S as a burst in phase A2 reading Qn/Kn saved in SBUF,
    instead of accumulating in 4 PSUM banks pinned across all of phase A.
    PSUM pools re-split (proj 3 / colS+out 3 / St 2) so phase A of batch
    b+1 and phase B of batch b touch disjoint pools and can overlap.
    Vt transpose issues move ACT -> SP.

    Projections must stay f32r: bf16 x/W gives 2.5e-2 rel err (fails the
    2e-2 gate) because the 2048-term col-logit sums amplify correlated
    weight quantization error; fp8 qk-projection gives 7e-2 (measured).

    29 = 28 with all PSUM drains on DVE/ACT (GPSIMD cannot access PSUM on
    real HW - neuronx-cc birverifier rejects it; CoreSim doesn't model
    that).  The 4 PE transposes of each xT / Qt / Kt chunk group land in
    one PSUM bank (disjoint 128-col slices, skip_group_check) and drain
    with a single wide DVE copy.  Pool keeps only SBUF->SBUF work
    (Vn8 cast, Acol normalize).
    31 = 29 + Qt/Kt merged into one QKt tile so all 8 q/k transposes of a
    token chunk share one PSUM bank and ONE DVE drain; oc PSUM copy moves
    to ACT.
    """
    import concourse.bass as bass
    import concourse.tile as tile
    from concourse import bacc, mybir
    from concourse.masks import make_identity

    f32 = mybir.dt.float32
    f32r = mybir.dt.float32r
    f8 = mybir.dt.float8e4
    # 34 = fp16 replaces f32r for W/xT (same 1 cyc/col PE rate as bf16 vs
    # f32r's ~274 ns/matmul stationary-reload tax; error measured identical
    # to f32r through both paths) and fp16 replaces bf16 everywhere 16-bit.
    if variant >= 34:
        f32r = mybir.dt.float16
        bf16 = mybir.dt.float16
    else:
        bf16 = mybir.dt.bfloat16
    AF = mybir.ActivationFunctionType
    AX = mybir.AxisListType
    DR = mybir.MatmulPerfMode.DoubleRow

    fp8col = variant == 25
    colS_dt = bf16 if variant >= 26 else f8
    qkt_via_transpose = variant >= 27
    colS_burst = variant >= 28
    # no PSUM access from Pool on real HW; 1 = batched bank drains,
    # 2 = alternating single DVE/ACT drains (fallback if HW disagrees with
    # the sim about reading a bank slice re-marked pending-zero)
    hw_legal = 0 if variant < 29 else (2 if variant == 30 else 1)
    precast = variant >= 35  # cast x to fp16 on Pool; 1 cyc/row transposes
    qk_merged = variant >= 31
    resplit = variant == 32  # transposes get their own PSUM pool
    slice_xt = variant >= 33  # xT as rolling 512-token slices + dbuf big tiles

    nc = bacc.Bacc("TRN2")
    x = nc.dram_tensor("x", [nb, N, E], f32, kind="ExternalInput")
    w = nc.dram_tensor("w", [E, 3 * E], f32, kind="ExternalInput")
    bvec = nc.dram_tensor("b", [3 * E], f32, kind="ExternalInput")
    y = nc.dram_tensor("y", [nb, N, E], f32, kind="ExternalOutput")

    with tile.TileContext(nc) as tc:
        with (
            tc.tile_pool(name="const", bufs=1) as constp,
            tc.tile_pool(name="xn", bufs=3) as xnp,
            tc.tile_pool(name="big", bufs=1) as bigp,
            tc.tile_pool(name="qkp", bufs=2) as qkpp,
            tc.tile_pool(name="step", bufs=2) as stepp,
            tc.tile_pool(name="stat", bufs=3) as statp,
            tc.tile_pool(name="outp", bufs=3) as outpp,
            tc.tile_pool(name="ps_proj", bufs=2 if resplit else (3 if colS_burst else 2), space="PSUM") as ps_proj,
            tc.tile_pool(name="ps_sc", bufs=2 if resplit else (3 if colS_burst else 4), space="PSUM") as ps_sc,
            tc.tile_pool(name="ps_s", bufs=2, space="PSUM") as ps_s,
            tc.tile_pool(name="ps_tps", bufs=2, space="PSUM") as ps_tps,
        ):
            # ---------------- constants ----------------
            W = constp.tile([128, ED, 3 * E], f32r)
            wv = w[:].rearrange("(k p) m -> p k m", p=128)
            for k in range(ED):
                for c in range(3):
                    wst = xnp.tile([128, E], f32, tag="xn", name=f"wst{k}_{c}")
                    nc.sync.dma_start(wst, wv[:, k, c * E : (c + 1) * E])
                    nc.vector.tensor_copy(W[:, k, c * E : (c + 1) * E], wst)

            b3 = bvec[:].rearrange("(c m) -> c m", m=E)
            bb = constp.tile([128, 3, E], bf16)
            nc.gpsimd.dma_start(
                bb, bass.AP(tensor=b3.tensor, offset=b3.offset, ap=[[0, 128]] + list(b3.ap))
            )
            bpart = constp.tile([128, 3 * ED], f32)
            nc.gpsimd.dma_start(bpart, bvec[:].rearrange("(c p) -> p c", p=128))

            ident = constp.tile([128, 128], f32)
            make_identity(nc, ident)
            ones8 = constp.tile([128, 2, 1], f8)
            nc.gpsimd.memset(ones8, 1.0)
            if qkt_via_transpose:
                identB = constp.tile([128, 128], bf16)
                make_identity(nc, identB)

            def batch_body():
              for b in range(nb):
                if not slice_xt:
                    xT = bigp.tile([128, ED, N], f32r, tag="xT")
                if qk_merged:
                    QKt = bigp.tile([128, 2, ED, N], f8, tag="QKt",
                                    bufs=2 if slice_xt else 1)
                    Qt, Kt = QKt[:, 0], QKt[:, 1]
                else:
                    Qt = bigp.tile([128, ED, N], f8, tag="Qt")
                    Kt = bigp.tile([128, ED, N], f8, tag="Kt")
                Vn8 = bigp.tile([128, NE, E], f8, tag="Vn8")
                Vnb = bigp.tile([128, NE, E], bf16, tag="Vnb")
                vt_dt = f8 if fp8col else bf16
                Vt = bigp.tile([128, ED, N], vt_dt, tag="Vt")
                Acol = bigp.tile([128, ED, E], bf16, tag="Acol",
                                 bufs=2 if slice_xt else 1)
                AcolT = bigp.tile([128, ED, E], vt_dt, tag="AcolT",
                                  bufs=2 if slice_xt else 1)
                if colS_burst:
                    Qn = bigp.tile([128, NE, E], bf16, tag="Qn")
                    Kn = bigp.tile([128, NE, E], bf16, tag="Kn")
                    scol_ps = None
                else:
                    scol_ps = [
                        ps_sc.tile([128, E], f32, tag="scps", name=f"scol{b}_{i}")
                        for i in range(ED)
                    ]

                # ---- phase A: load x, build xT, projections, col-S accum
                qp = kp = None
                for j in range(NE):
                    s_idx = j // ED
                    xoff, roff = j * 128, s_idx * 512
                    if slice_xt:
                        if j % ED == 0:
                            xT = qkpp.tile(
                                [128, ED, 512], f32r, tag="xsl", name=f"xsl{b}_{s_idx}"
                            )
                        xoff = (j % ED) * 128
                    xn = xnp.tile([128, E], f32, tag="xn")
                    nc.sync.dma_start(xn, x[b, j * 128 : (j + 1) * 128, :])
                    if precast:
                        xnh = xnp.tile([128, E], bf16, tag="xnh")
                        nc.gpsimd.tensor_copy(xnh, xn)
                        xn = xnh
                    t_dt, t_id = (bf16, identB) if precast else (f32, ident)
                    if hw_legal == 1:
                        tpool = ps_tps if resplit else ps_proj
                        tpsx = tpool.tile([128, ED, 128], t_dt, tag="tps" if resplit else "ps", name=f"tpsx{b}_{j}")
                        for k in range(ED):
                            nc.tensor.matmul(
                                tpsx[:, k, :],
                                xn[:, k * 128 : (k + 1) * 128],
                                t_id,
                                is_transpose=True,
                                skip_group_check=True,
                            )
                        nc.vector.tensor_copy(xT[:, :, xoff : xoff + 128], tpsx)
                    elif hw_legal:
                        for k in range(ED):
                            tps = ps_proj.tile([128, 128], f32, tag="ps")
                            nc.tensor.transpose(tps, xn[:, k * 128 : (k + 1) * 128], ident)
                            eng = nc.vector if k % 2 == 0 else nc.scalar
                            if k % 2 == 0:
                                eng.tensor_copy(xT[:, k, xoff : xoff + 128], tps)
                            else:
                                eng.copy(xT[:, k, xoff : xoff + 128], tps)
                    else:
                        for k in range(ED):
                            tps = ps_proj.tile([128, 128], f32, tag="ps")
                            nc.tensor.transpose(tps, xn[:, k * 128 : (k + 1) * 128], ident)
                            nc.gpsimd.tensor_copy(xT[:, k, xoff : xoff + 128], tps)

                    # natural-layout q, k (pair-staged or SBUF-resident), v
                    if colS_burst:
                        qdst, kdst = Qn[:, j, :], Kn[:, j, :]
                    else:
                        if j % 2 == 0:
                            qp = qkpp.tile([128, 2, E], colS_dt, tag="qp", name=f"qp{b}_{j}")
                            kp = qkpp.tile([128, 2, E], colS_dt, tag="kp", name=f"kp{b}_{j}")
                        qdst, kdst = qp[:, j % 2, :], kp[:, j % 2, :]
                    for dst, ci in (
                        (qdst, 0),
                        (kdst, 1),
                        (Vnb[:, j, :], 2),
                    ):
                        pp = ps_proj.tile([128, E], f32, tag="ps")
                        for k in range(ED):
                            nc.tensor.matmul(
                                pp,
                                xT[:, k, xoff : xoff + 128],
                                W[:, k, ci * E : (ci + 1) * E],
                                start=(k == 0),
                                stop=(k == ED - 1),
                            )
                        nc.vector.tensor_add(dst, pp, bb[:, ci, :])
                    nc.gpsimd.tensor_copy(Vn8[:, j, :], Vnb[:, j, :])
                    if not fp8col:
                        vt_eng = nc.sync if colS_burst else nc.scalar
                        vt_eng.dma_start_transpose(
                            Vt[:, :, j * 128 : (j + 1) * 128], Vnb[:, j, :]
                        )

                    # col-S accumulation, every second chunk
                    if not colS_burst and j % 2 == 1:
                        if colS_dt is f8:
                            for i in range(ED):
                                nc.tensor.matmul(
                                    scol_ps[i],
                                    qp[:, :, i * 128 : (i + 1) * 128],
                                    kp,
                                    start=(j == 1),
                                    stop=(j == NE - 1),
                                    perf_mode=DR,
                                )
                        else:
                            for jj in range(2):
                                for i in range(ED):
                                    nc.tensor.matmul(
                                        scol_ps[i],
                                        qp[:, jj, i * 128 : (i + 1) * 128],
                                        kp[:, jj, :],
                                        start=(j == 1 and jj == 0),
                                        stop=(j == NE - 1 and jj == 1),
                                    )

                    # Qt/Kt transposed layouts
                    if qkt_via_transpose and qk_merged:
                        qkpool = ps_tps if resplit else ps_proj
                        psqk = qkpool.tile(
                            [128, 2, ED, 128], bf16, tag="tps" if resplit else "ps", name=f"psqk{b}_{j}"
                        )
                        for ci, src in ((0, qdst), (1, kdst)):
                            for i in range(ED):
                                nc.tensor.matmul(
                                    psqk[:, ci, i, :],
                                    src[:, i * 128 : (i + 1) * 128],
                                    identB,
                                    is_transpose=True,
                                    skip_group_check=True,
                                )
                        nc.vector.tensor_copy(QKt[:, :, :, j * 128 : (j + 1) * 128], psqk)
                    elif qkt_via_transpose:
                        for (dst, src, ci) in ((Qt, qdst, 0), (Kt, kdst, 1)):
                            if hw_legal == 1:
                                psB = ps_proj.tile(
                                    [128, ED, 128], bf16, tag="ps", name=f"psB{b}_{j}_{ci}"
                                )
                                for i in range(ED):
                                    nc.tensor.matmul(
                                        psB[:, i, :],
                                        src[:, i * 128 : (i + 1) * 128],
                                        identB,
                                        is_transpose=True,
                                        skip_group_check=True,
                                    )
                                nc.vector.tensor_copy(
                                    dst[:, :, j * 128 : (j + 1) * 128], psB
                                )
                            else:
                                for i in range(ED):
                                    psB = ps_proj.tile(
                                        [128, 128], bf16, tag="ps", name=f"psB{b}_{j}_{ci}_{i}"
                                    )
                                    nc.tensor.transpose(
                                        psB, src[:, i * 128 : (i + 1) * 128], identB
                                    )
                                    if hw_legal:
                                        if i % 2 == 0:
                                            nc.vector.tensor_copy(
                                                dst[:, i, j * 128 : (j + 1) * 128], psB
                                            )
                                        else:
                                            nc.scalar.copy(
                                                dst[:, i, j * 128 : (j + 1) * 128], psB
                                            )
                                    else:
                                        nc.gpsimd.tensor_copy(
                                            dst[:, i, j * 128 : (j + 1) * 128], psB
                                        )
                    elif j % ED == ED - 1:
                        sl = slice(s_idx * 512, (s_idx + 1) * 512)
                        for dst, ci in ((Qt, 0), (Kt, 1)):
                            for i in range(ED):
                                pp = ps_proj.tile([128, E], f32, tag="ps")
                                for k in range(ED):
                                    nc.tensor.matmul(
                                        pp,
                                        W[:, k, ci * E + i * 128 : ci * E + (i + 1) * 128],
                                        xT[:, k, roff : roff + 512],
                                        start=(k == 0),
                                        stop=(k == ED - 1),
                                    )
                                nc.scalar.activation(
                                    out=dst[:, i, sl],
                                    in_=pp,
                                    func=AF.Identity,
                                    bias=bpart[:, ci * ED + i : ci * ED + i + 1],
                                )

                # ---- phase A2: col softmax (max-sub) + transpose of A
                cstat = statp.tile([128, 3 * ED], f32, tag="cstat")
                for i in range(ED):
                    if colS_burst:
                        scps = ps_sc.tile([128, E], f32, tag="scps", name=f"scol{b}_{i}")
                        for jc in range(NE):
                            nc.tensor.matmul(
                                scps,
                                Qn[:, jc, i * 128 : (i + 1) * 128],
                                Kn[:, jc, :],
                                start=(jc == 0),
                                stop=(jc == NE - 1),
                            )
                    else:
                        scps = scol_ps[i]
                    nm = cstat[:, 2 * ED + i : 2 * ED + i + 1]
                    nc.vector.reduce_max(nm, scps, axis=AX.X, negate=True)
                    nc.vector.tensor_scalar_mul(nm, nm, SCALE)
                    nc.scalar.activation(
                        out=Acol[:, i, :],
                        in_=scps,
                        func=AF.Exp,
                        scale=SCALE,
                        bias=nm,
                        accum_out=cstat[:, i : i + 1],
                    )
                nc.vector.reciprocal(cstat[:, ED : 2 * ED], cstat[:, 0:ED])
                norm_eng = nc.gpsimd if hw_legal else nc.vector
                for i in range(ED):
                    norm_eng.tensor_scalar_mul(
                        Acol[:, i, :], Acol[:, i, :], cstat[:, ED + i : ED + i + 1]
                    )
                    if not fp8col:
                        nc.scalar.dma_start_transpose(
                            AcolT[:, :, i * 128 : (i + 1) * 128], Acol[:, i, :]
                        )

                # ---- phase B: St-direct row attention + merged output
                for s in range(ED):
                    ssl = slice(s * 512, (s + 1) * 512)
                    StE = stepp.tile([128, NE, 512], f8, tag="ste", name=f"ste{b}_{s}")
                    for m in range(NE):
                        sps = ps_s.tile([128, 512], f32, tag="s")
                        for kk in range(2):
                            nc.tensor.matmul(
                                sps,
                                Kt[:, 2 * kk : 2 * kk + 2, m * 128 : (m + 1) * 128],
                                Qt[:, 2 * kk : 2 * kk + 2, ssl],
                                start=(kk == 0),
                                stop=(kk == 1),
                                perf_mode=DR,
                            )
                        nc.scalar.activation(
                            out=StE[:, m, :], in_=sps, func=AF.Exp, scale=SCALE
                        )
                    rs_ps = ps_sc.tile([128, 4], f32, tag="scps", name=f"rs{b}_{s}")
                    for jj in range(4):
                        for mm in range(8):
                            nc.tensor.matmul(
                                rs_ps[:, jj : jj + 1],
                                StE[:, 2 * mm : 2 * mm + 2, jj * 128 : (jj + 1) * 128],
                                ones8,
                                start=(mm == 0),
                                stop=(mm == 7),
                                perf_mode=DR,
                                skip_group_check=True,
                            )
                    rstat = statp.tile([128, 4], f32, tag="rstat")
                    nc.vector.reciprocal(rstat, rs_ps)
                    for jj in range(4):
                        j = s * 4 + jj
                        jsl = slice(j * 128, (j + 1) * 128)
                        po = ps_sc.tile([128, E], f32, tag="scps")
                        for mm in range(8):
                            nc.tensor.matmul(
                                po,
                                StE[:, 2 * mm : 2 * mm + 2, jj * 128 : (jj + 1) * 128],
                                Vn8[:, 2 * mm : 2 * mm + 2, :],
                                start=(mm == 0),
                                stop=(mm == 7),
                                perf_mode=DR,
                            )
                        oc = ps_sc.tile([128, E], f32, tag="scps")
                        if fp8col:
                            for c in range(2):
                                nc.tensor.matmul(
                                    oc,
                                    Vt[:, 2 * c : 2 * c + 2, jsl],
                                    AcolT[:, 2 * c : 2 * c + 2, :],
                                    start=(c == 0),
                                    stop=(c == 1),
                                    perf_mode=DR,
                                )
                        else:
                            for c in range(ED):
                                nc.tensor.matmul(
                                    oc,
                                    Vt[:, c, jsl],
                                    AcolT[:, c, :],
                                    start=(c == 0),
                                    stop=(c == ED - 1),
                                )
                        octmp = outpp.tile([128, E], f32, tag="octmp")
                        if qk_merged:
                            nc.scalar.copy(octmp, oc)
                        elif hw_legal:
                            nc.vector.tensor_copy(octmp, oc)
                        else:
                            nc.gpsimd.tensor_copy(octmp, oc)
                        ot = outpp.tile([128, E], f32, tag="ot")
                        nc.vector.scalar_tensor_tensor(
                            ot,
                            po,
                            rstat[:, jj : jj + 1],
                            octmp,
                            op0=mybir.AluOpType.mult,
                            op1=mybir.AluOpType.add,
                        )
                        nc.sync.dma_start(y[b, jsl, :], ot)

            if reps == 1:
                batch_body()
            else:
                with tc.For_i(0, reps, 1):
                    batch_body()

    nc.compile()
    return nc


BEST_VARIANT = 35


def kernel(x, w_qkv, b_qkv):
    from concourse.bass_utils import run_bass_kernel_spmd

    nc = build_nc(NB, BEST_VARIANT)
    in_maps = make_in_maps(x, w_qkv, b_qkv)
    res = run_bass_kernel_spmd(nc, in_maps, core_ids=list(range(NCORES)))
    out = np.empty((B, N, E), dtype=np.float32)
    for c in range(NCORES):
        out[c * NB : (c + 1) * NB] = res.results[c]["y"]
    return out



# revision 49
# speedup vs baseline: 1.1864x; 1.1567x over previous
"""AxialSelfAttention Trainium2 Bass kernel.

Reference computation (per batch b):
    xs  = x[b] reshaped [N=2048, E=512]
    qkv = xs @ W + bias                      # [N, 3E]
    q, k, v = split(qkv)
    row:  P = softmax(q @ k.T / sqrt(E));  out_row = P @ v
    col:  A = softmax(q.T @ k / sqrt(E));  out_col = v @ A.T
    out = out_row + out_col                  # [N, E]

Sharding: data-parallel over batch B=32 across 8 cores (4 batches/core).

Best variant (36, see build_nc_v24): St-direct row attention + fp8
DoubleRow + fp16 projections.  Per batch:
  - x chunks are cast to fp16 on Pool, PE-transposed (1 cyc/row) into
    rolling 512-token xT slices; q/k/v projections are fp16 matmuls from
    xT.  fp16 is the precision sweet spot: bf16 projections fail the gate
    (2.5e-2 - the 2048-term col-logit sums amplify correlated weight
    quantization error 6x) while fp16 is indistinguishable from f32r
    (measured) and avoids f32r's ~80 ns/matmul stationary-reload tax.
  - Natural q/k (fp16) are saved in SBUF; col-S runs as a burst of fp16
    matmuls in rotating PSUM banks, then max-subtracted softmax -> Acol,
    DMA-XBAR-transposed to AcolT.
  - Qt/Kt come from fp16 PE transposes of q/k, drained fp8 into one merged
    QKt tile (all 8 transposes of a chunk share one PSUM bank + one DVE
    drain).
  - Row attention computes S TRANSPOSED (stat=Kt chunk, mov=Qt slice) in
    fp8 DoubleRow; exp(St) lands directly in the PV stationary layout
    (fp8, unnormalized - row logits are in [-1.3, 1.0] so exp fits fp8).
    Row sums come from ones-vector DoubleRow matmuls; 1/rowsum is applied
    in the final DVE merge (out = po * rinv + out_col).
  - PV and colout accumulate in separate PSUM tiles; merged on DVE.
Measured on HW (8 cores, reps-slope, matched-epoch A/B): v13 baseline
1.10 ms -> v33 0.84 ms -> v35/v36 ~0.72-0.74 ms per invocation (v36 adds
a double-buffered Vt so batch b+1's V transposes overlap batch b's col
output); rel err 1.2e-3.
"""

import sys

for _p in ("/opt/trn_rl_repo", "/root/.axon_site/_ro/trn_rl_repo"):
    if _p not in sys.path:
        sys.path.append(_p)

import numpy as np

B, N, E = 32, 2048, 512
NCORES = 8
NB = B // NCORES  # batches per core
NE = N // 128  # 16 token chunks
ED = E // 128  # 4 feature chunks
SCALE = 1.0 / float(np.sqrt(E))

_NC_CACHE = {}


def build_nc(nb=NB, variant=13, reps=1):
    """Build (once) the single-core Bass module processing nb batches.

    variant 1: all six projection layouts via fp32r matmuls from xT.
    variant 2: like 1, but Vt comes from a bf16 DMA(XBAR)-transpose of V
               instead of its own matmul projection (-64 matmuls/batch).
    variant 3: like 2, and Qt/Kt also come from DMA-transposes of the bf16
               natural q/k (row-attention S then runs in bf16;
               -128 more matmuls/batch).
    variant 4: like 3, plus V/Acol/AcolT double-buffered across batches so
               batch b+1's projection phase (PE) can stream while batch b's
               row attention still reads V.
    variant 5: variant 2 + early transposes: exp quarters are transposed
               unnormalized as soon as they exist, and the 1/rowsum lands on
               the PV output (per-partition ACT scale) instead of on P~;
               PV and the col output use separate PSUM tiles.
    variant 6: variant 5 + the double-buffering of variant 4.
    variant 7: variant 6 + 3-deep prow pool.
    """
    FEAT = {
        1: set(),
        2: {"vt_dma"},
        3: {"vt_dma", "qkt_dma"},
        4: {"vt_dma", "qkt_dma", "dbuf"},
        5: {"vt_dma", "early_t"},
        6: {"vt_dma", "early_t", "dbuf"},
        7: {"vt_dma", "early_t", "dbuf", "prow3"},
        8: {"vt_dma", "spread"},
        9: {"vt_dma", "early_t", "spread"},
        10: {"vt_dma", "late_norm", "act_drain"},
        11: {"vt_dma", "act_drain"},
        13: {"vt_dma", "late_norm", "pe_pt", "dve_merge"},
        14: {"vt_dma", "late_norm", "dve_merge"},
        # timing-only diagnostics (wrong results): fake / absent pt transposes
        15: {"vt_dma", "fake_t"},
        16: {"vt_dma", "no_t"},
        17: {"late_norm", "pe_pt", "dve_merge"},
        18: {"vt_dma", "late_norm", "pe_pt", "dve_merge", "prow3"},
        20: {"vt_dma", "late_norm", "pe_pt", "dve_merge", "psum2"},
        22: {"late_norm", "pe_pt", "dve_merge", "f32r_pv", "slice_xt"},
        23: {"late_norm", "pe_pt", "dve_merge", "f32r_pv", "slice_xt", "prow3"},
    }
    if (nb, variant, reps) in _NC_CACHE:
        return _NC_CACHE[(nb, variant, reps)]
    if variant >= 24:
        nc = build_nc_v24(nb, variant, reps)
        _NC_CACHE[(nb, variant, reps)] = nc
        return nc
    feat = FEAT[variant]

    import concourse.bass as bass
    import concourse.tile as tile
    from concourse import bacc, mybir
    from concourse.masks import make_identity

    f32 = mybir.dt.float32
    f32r = mybir.dt.float32r
    bf16 = mybir.dt.bfloat16
    AF = mybir.ActivationFunctionType
    AX = mybir.AxisListType

    nc = bacc.Bacc("TRN2")
    x = nc.dram_tensor("x", [nb, N, E], f32, kind="ExternalInput")
    w = nc.dram_tensor("w", [E, 3 * E], f32, kind="ExternalInput")
    bvec = nc.dram_tensor("b", [3 * E], f32, kind="ExternalInput")
    y = nc.dram_tensor("y", [nb, N, E], f32, kind="ExternalOutput")

    with tile.TileContext(nc) as tc:
        with (
            tc.tile_pool(name="const", bufs=1) as constp,
            tc.tile_pool(name="xn", bufs=2 if ("prow3" in feat or "slice_xt" in feat) else 3) as xnp,
            tc.tile_pool(name="big", bufs=1) as bigp,
            tc.tile_pool(name="xsl", bufs=2) as xslp,
            tc.tile_pool(name="qkn", bufs=3) as qknp,
            tc.tile_pool(name="prow", bufs=3 if "prow3" in feat else 2) as prowp,
            tc.tile_pool(name="stat", bufs=3) as statp,
            tc.tile_pool(name="outp", bufs=2) as outpp,
            tc.tile_pool(name="ps_proj", bufs=2, space="PSUM") as ps_proj,
            tc.tile_pool(name="ps_sc", bufs=4, space="PSUM") as ps_sc,
            tc.tile_pool(name="ps_s", bufs=2, space="PSUM") as ps_s,
        ):
            # ---------------- constants ----------------
            # W lands as float32r (rounded by the DVE copy) so fp32r matmuls
            # accept it; staged through the small xn pool to save SBUF.
            W = constp.tile([128, ED, 3 * E], f32r)
            wv = w[:].rearrange("(k p) m -> p k m", p=128)
            for k in range(ED):
                for c in range(3):
                    wst = xnp.tile([128, E], f32, tag="xn", name=f"wst{k}_{c}")
                    nc.sync.dma_start(wst, wv[:, k, c * E : (c + 1) * E])
                    nc.vector.tensor_copy(W[:, k, c * E : (c + 1) * E], wst)

            # bias broadcast across partitions (for [n-part, e] layouts)
            b3 = bvec[:].rearrange("(c m) -> c m", m=E)
            bb = constp.tile([128, 3, E], bf16)
            nc.gpsimd.dma_start(
                bb, bass.AP(tensor=b3.tensor, offset=b3.offset, ap=[[0, 128]] + list(b3.ap))
            )
            # bias per partition (for [e-part, n] layouts): bpart[p, c] = b[c*128+p]
            bpart = constp.tile([128, 3 * ED], f32)
            nc.gpsimd.dma_start(bpart, bvec[:].rearrange("(c p) -> p c", p=128))

            ident = constp.tile([128, 128], f32)
            make_identity(nc, ident)
            identB = constp.tile([128, 128], bf16)
            make_identity(nc, identB)

            def batch_body():
              for b in range(nb):
                qkt_dt = bf16 if "qkt_dma" in feat else f32r
                vn_dt = f32r if "f32r_pv" in feat else bf16
                dbufs = 2 if "dbuf" in feat else 1
                slice_xt = "slice_xt" in feat
                if not slice_xt:
                    xT = bigp.tile([128, ED, N], f32r, tag="xT")
                Qt = bigp.tile([128, ED, N], qkt_dt, tag="Qt")
                Kt = bigp.tile([128, ED, N], qkt_dt, tag="Kt")
                Vn = bigp.tile([128, NE, E], vn_dt, tag="Vn", bufs=dbufs)
                Vt = bigp.tile([128, ED, N], bf16, tag="Vt")
                Acol = bigp.tile([128, ED, E], bf16, tag="Acol", bufs=dbufs)
                AcolT = bigp.tile([128, ED, E], bf16, tag="AcolT", bufs=dbufs)
                scol_ps = [
                    ps_sc.tile([128, E], f32, tag="scps", name=f"scol{b}_{i}")
                    for i in range(ED)
                ]

                # ---- phase A: load x, build xT, projections, col-S accumulation
                for j in range(NE):
                    s_idx, jj = j // ED, j % ED
                    if slice_xt:
                        if jj == 0:
                            xT = xslp.tile(
                                [128, ED, ED * 128], f32r, tag="xsl", name=f"xsl{b}_{s_idx}"
                            )
                        xoff, roff = jj * 128, 0
                    else:
                        xoff, roff = j * 128, s_idx * 512
                    xn = xnp.tile([128, E], f32, tag="xn")
                    nc.sync.dma_start(xn, x[b, j * 128 : (j + 1) * 128, :])
                    for k in range(ED):
                        tps = ps_proj.tile([128, 128], f32, tag="ps")
                        nc.tensor.transpose(tps, xn[:, k * 128 : (k + 1) * 128], ident)
                        nc.vector.tensor_copy(xT[:, k, xoff : xoff + 128], tps)

                    # natural-layout q, k, v for this token chunk
                    qn = qknp.tile([128, E], bf16, tag="qn")
                    kn = qknp.tile([128, E], bf16, tag="kn")
                    for dst, ci in ((qn, 0), (kn, 1), (Vn[:, j, :], 2)):
                        pp = ps_proj.tile([128, E], f32, tag="ps")
                        for k in range(ED):
                            nc.tensor.matmul(
                                pp,
                                xT[:, k, xoff : xoff + 128],
                                W[:, k, ci * E : (ci + 1) * E],
                                start=(k == 0),
                                stop=(k == ED - 1),
                            )
                        nc.vector.tensor_add(dst, pp, bb[:, ci, :])

                    # col-attention S accumulation: S_col[d,e] += q_j.T @ k_j
                    for i in range(ED):
                        nc.tensor.matmul(
                            scol_ps[i],
                            qn[:, i * 128 : (i + 1) * 128],
                            kn,
                            start=(j == 0),
                            stop=(j == NE - 1),
                        )

                    # bf16 transposed layouts via the DMA XBAR (free wrt PE)
                    jsl = slice(j * 128, (j + 1) * 128)
                    if "vt_dma" in feat:
                        nc.scalar.dma_start_transpose(Vt[:, :, jsl], Vn[:, j, :])
                    if "qkt_dma" in feat:
                        nc.scalar.dma_start_transpose(Qt[:, :, jsl], qn)
                        nc.scalar.dma_start_transpose(Kt[:, :, jsl], kn)

                    # transposed-layout projections, one 512-token slice at a time
                    if "qkt_dma" in feat:
                        tproj = ()
                    elif "vt_dma" in feat:
                        tproj = ((Qt, 0), (Kt, 1))
                    else:
                        tproj = ((Qt, 0), (Kt, 1), (Vt, 2))
                    if j % ED == ED - 1 and tproj:
                        sl = slice(s_idx * 512, (s_idx + 1) * 512)
                        for dst, ci in tproj:
                            for i in range(ED):
                                pp = ps_proj.tile([128, E], f32, tag="ps")
                                for k in range(ED):
                                    nc.tensor.matmul(
                                        pp,
                                        W[:, k, ci * E + i * 128 : ci * E + (i + 1) * 128],
                                        xT[:, k, roff : roff + 512],
                                        start=(k == 0),
                                        stop=(k == ED - 1),
                                    )
                                if "act_drain" in feat:
                                    nc.scalar.activation(
                                        out=dst[:, i, sl],
                                        in_=pp,
                                        func=AF.Identity,
                                        bias=bpart[:, ci * ED + i : ci * ED + i + 1],
                                    )
                                else:
                                    nc.vector.tensor_scalar_add(
                                        dst[:, i, sl], pp, bpart[:, ci * ED + i : ci * ED + i + 1]
                                    )

                # ---- phase A2: col softmax + transpose of A
                # col logits are O(+-600): subtract the per-row max (as an ACT
                # bias of -max*SCALE) before exp, unlike the row path.
                cstat = statp.tile([128, 3 * ED], f32, tag="cstat")
                for i in range(ED):
                    nm = cstat[:, 2 * ED + i : 2 * ED + i + 1]
                    nc.vector.reduce_max(nm, scol_ps[i], axis=AX.X, negate=True)
                    nc.vector.tensor_scalar_mul(nm, nm, SCALE)
                    nc.scalar.activation(
                        out=Acol[:, i, :],
                        in_=scol_ps[i],
                        func=AF.Exp,
                        scale=SCALE,
                        bias=nm,
                        accum_out=cstat[:, i : i + 1],
                    )
                nc.vector.reciprocal(cstat[:, ED : 2 * ED], cstat[:, 0:ED])
                for i in range(ED):
                    nc.vector.tensor_scalar_mul(
                        Acol[:, i, :], Acol[:, i, :], cstat[:, ED + i : ED + i + 1]
                    )
                    nc.scalar.dma_start_transpose(
                        AcolT[:, :, i * 128 : (i + 1) * 128], Acol[:, i, :]
                    )

                # ---- phase B: row attention + merged output, per token chunk
                early_t = "early_t" in feat
                late_norm = "late_norm" in feat
                spread = "spread" in feat
                for j in range(NE):
                    teng = (nc.sync if j % 2 else nc.scalar) if spread else nc.scalar
                    yeng = (nc.scalar if j % 2 else nc.sync) if spread else nc.sync
                    pt = prowp.tile([128, N], bf16, tag="pt")
                    ptT = prowp.tile(
                        [128, NE, 128], f32r if "f32r_pv" in feat else bf16, tag="ptT"
                    )
                    rstat = statp.tile([128, 8], f32, tag="rstat")
                    for q in range(4):
                        sps = ps_s.tile([128, 512], f32, tag="s")
                        for k in range(ED):
                            nc.tensor.matmul(
                                sps,
                                Qt[:, k, j * 128 : (j + 1) * 128],
                                Kt[:, k, q * 512 : (q + 1) * 512],
                                start=(k == 0),
                                stop=(k == ED - 1),
                            )
                        nc.scalar.activation(
                            out=pt[:, q * 512 : (q + 1) * 512],
                            in_=sps,
                            func=AF.Exp,
                            scale=SCALE,
                            accum_out=rstat[:, q : q + 1],
                        )
                        if early_t:
                            # transpose the unnormalized quarter right away;
                            # 1/rowsum is applied to the PV output instead
                            teng.dma_start_transpose(
                                ptT[:, 4 * q : 4 * q + 4, :],
                                pt[:, q * 512 : (q + 1) * 512],
                            )
                        if "pe_pt" in feat:
                            for t in range(4):
                                m = 4 * q + t
                                psB = ps_proj.tile(
                                    [128, 128], bf16, tag="ps", name=f"psB{b}_{j}_{m}"
                                )
                                nc.tensor.transpose(
                                    psB, pt[:, m * 128 : (m + 1) * 128], identB
                                )
                                nc.vector.tensor_copy(ptT[:, m, :], psB)
                    nc.vector.reduce_sum(rstat[:, 4:5], rstat[:, 0:4], axis=AX.X)
                    nc.vector.reciprocal(rstat[:, 5:6], rstat[:, 4:5])
                    if late_norm:
                        if "pe_pt" not in feat:
                            teng.dma_start_transpose(ptT, pt)
                    elif not early_t:
                        nc.vector.tensor_scalar_mul(pt, pt, rstat[:, 5:6])
                        if "fake_t" in feat:
                            teng.dma_start(ptT.rearrange("p a b -> p (a b)"), pt)
                        elif "no_t" in feat:
                            nc.vector.tensor_copy(ptT[:, 0, :], pt[:, :128])
                        else:
                            teng.dma_start_transpose(ptT, pt)

                    po = ps_sc.tile([128, E], f32, tag="scps")
                    for m in range(NE):
                        nc.tensor.matmul(
                            po,
                            ptT[:, m, :],
                            Vn[:, m, :],
                            start=(m == 0),
                            stop=((early_t or late_norm) and m == NE - 1),
                        )
                    ot = outpp.tile([128, E], f32, tag="ot")
                    if early_t or late_norm:
                        oc = ps_sc.tile([128, E], f32, tag="scps")
                        for c in range(ED):
                            nc.tensor.matmul(
                                oc,
                                Vt[:, c, j * 128 : (j + 1) * 128],
                                AcolT[:, c, :],
                                start=(c == 0),
                                stop=(c == ED - 1),
                            )
                        if "dve_merge" in feat:
                            if "psum2" in feat:
                                nc.vector.scalar_tensor_tensor(
                                    ot,
                                    po,
                                    rstat[:, 5:6],
                                    oc,
                                    op0=mybir.AluOpType.mult,
                                    op1=mybir.AluOpType.add,
                                )
                            else:
                                octmp = outpp.tile([128, E], f32, tag="octmp")
                                nc.vector.tensor_copy(octmp, oc)
                                nc.vector.scalar_tensor_tensor(
                                    ot,
                                    po,
                                    rstat[:, 5:6],
                                    octmp,
                                    op0=mybir.AluOpType.mult,
                                    op1=mybir.AluOpType.add,
                                )
                        else:
                            nc.scalar.activation(
                                out=ot, in_=po, func=AF.Copy, scale=rstat[:, 5:6]
                            )
                            nc.vector.tensor_add(ot, ot, oc)
                    else:
                        for c in range(ED):
                            nc.tensor.matmul(
                                po,
                                Vt[:, c, j * 128 : (j + 1) * 128],
                                AcolT[:, c, :],
                                start=False,
                                stop=(c == ED - 1),
                            )
                        nc.vector.tensor_copy(ot, po)
                    yeng.dma_start(y[b, j * 128 : (j + 1) * 128, :], ot)

            if reps == 1:
                batch_body()
            else:
                with tc.For_i(0, reps, 1):
                    batch_body()

    nc.compile()
    _NC_CACHE[(nb, variant, reps)] = nc
    return nc


def make_in_maps(x, w_qkv, b_qkv):
    xs = np.ascontiguousarray(np.asarray(x, dtype=np.float32)).reshape(B, N, E)
    w = np.ascontiguousarray(np.asarray(w_qkv, dtype=np.float32))
    bq = np.ascontiguousarray(np.asarray(b_qkv, dtype=np.float32))
    return [
        {"x": np.ascontiguousarray(xs[c * NB : (c + 1) * NB]), "w": w, "b": bq}
        for c in range(NCORES)
    ]


def build_nc_v24(nb, variant, reps):
    """St-direct + fp8 DoubleRow redesign.

    Key differences vs v13:
      - Row attention computes S TRANSPOSED (St[m-part, j-free]) directly:
        stat=Kt chunk, mov=Qt slice.  exp(St) goes straight to the PV
        stationary layout, eliminating all 256/batch PE transposes of P and
        their 256 DVE PSUM drains.  Row sums come from tiny ones-vector
        matmuls accumulating [128,1] PSUM; 1/rowsum lands on the PV output
        via the DVE merge (late_norm).
      - Row-path matmuls (St, PV, colS) run in fp8e4 with DoubleRow perf
        mode: contraction pairs two 128-partition chunks per instruction.
        Row logits are in [-1.3, 1.0] so exp(s) in [0.27, 2.6] is perfectly
        fp8-representable unnormalized (measured on the reference).
      - Col path (max-sub softmax, colout matmuls) stays bf16: col logits
        are O(+-200) and max-dominated, so fp8 there is risky.
      - Projections stay f32r (fp8 W error is correlated across tokens and
        amplifies through the 2048-term col-logit sums).
      - PSUM drains spread across DVE / ACT / Pool to keep all engines
        under the PE time.

    variants: 24 = base; 25 = 24 + fp8 colout (Vt/AcolT fp8);
    26 = 24 but col-S operands bf16 (no DoubleRow there) — fp8 col logits
    cost 1.4e-2 rel err (max-dominated softmax amplifies logit noise,
    measured), while fp8 anywhere on the row path costs < 7e-4.
    27 = 26 + Qt/Kt from PE transposes of the bf16 qn/kn (Pool fp8 drains)
    instead of their own f32r projection passes (-20.5us/batch PE).
    28 = 27 + col-S as a burst in phase A2 reading Qn/Kn saved in SBUF,
    instead of accumulating in 4 PSUM banks pinned across all of phase A.
    PSUM pools re-split (proj 3 / colS+out 3 / St 2) so phase A of batch
    b+1 and phase B of batch b touch disjoint pools and can overlap.
    Vt transpose issues move ACT -> SP.

    Projections must stay f32r: bf16 x/W gives 2.5e-2 rel err (fails the
    2e-2 gate) because the 2048-term col-logit sums amplify correlated
    weight quantization error; fp8 qk-projection gives 7e-2 (measured).

    29 = 28 with all PSUM drains on DVE/ACT (GPSIMD cannot access PSUM on
    real HW - neuronx-cc birverifier rejects it; CoreSim doesn't model
    that).  The 4 PE transposes of each xT / Qt / Kt chunk group land in
    one PSUM bank (disjoint 128-col slices, skip_group_check) and drain
    with a single wide DVE copy.  Pool keeps only SBUF->SBUF work
    (Vn8 cast, Acol normalize).
    31 = 29 + Qt/Kt merged into one QKt tile so all 8 q/k transposes of a
    token chunk share one PSUM bank and ONE DVE drain; oc PSUM copy moves
    to ACT.
    """
    import concourse.bass as bass
    import concourse.tile as tile
    from concourse import bacc, mybir
    from concourse.masks import make_identity

    f32 = mybir.dt.float32
    f32r = mybir.dt.float32r
    f8 = mybir.dt.float8e4
    # 34 = fp16 replaces f32r for W/xT (same 1 cyc/col PE rate as bf16 vs
    # f32r's ~274 ns/matmul stationary-reload tax; error measured identical
    # to f32r through both paths) and fp16 replaces bf16 everywhere 16-bit.
    if variant >= 34:
        f32r = mybir.dt.float16
        bf16 = mybir.dt.float16
    else:
        bf16 = mybir.dt.bfloat16
    AF = mybir.ActivationFunctionType
    AX = mybir.AxisListType
    DR = mybir.MatmulPerfMode.DoubleRow

    fp8col = variant == 25
    colS_dt = bf16 if variant >= 26 else f8
    qkt_via_transpose = variant >= 27
    colS_burst = variant >= 28
    # no PSUM access from Pool on real HW; 1 = batched bank drains,
    # 2 = alternating single DVE/ACT drains (fallback if HW disagrees with
    # the sim about reading a bank slice re-marked pending-zero)
    hw_legal = 0 if variant < 29 else (2 if variant == 30 else 1)
    precast = variant >= 35  # cast x to fp16 on Pool; 1 cyc/row transposes
    dbuf_vt = variant >= 36  # fp16 freed ~20KB/partition: double-buffer Vt
    qk_merged = variant >= 31
    resplit = variant == 32  # transposes get their own PSUM pool
    slice_xt = variant >= 33  # xT as rolling 512-token slices + dbuf big tiles

    nc = bacc.Bacc("TRN2")
    x = nc.dram_tensor("x", [nb, N, E], f32, kind="ExternalInput")
    w = nc.dram_tensor("w", [E, 3 * E], f32, kind="ExternalInput")
    bvec = nc.dram_tensor("b", [3 * E], f32, kind="ExternalInput")
    y = nc.dram_tensor("y", [nb, N, E], f32, kind="ExternalOutput")

    with tile.TileContext(nc) as tc:
        with (
            tc.tile_pool(name="const", bufs=1) as constp,
            tc.tile_pool(name="xn", bufs=4 if variant >= 36 else 3) as xnp,
            tc.tile_pool(name="big", bufs=1) as bigp,
            tc.tile_pool(name="qkp", bufs=2) as qkpp,
            tc.tile_pool(name="step", bufs=2) as stepp,
            tc.tile_pool(name="stat", bufs=3) as statp,
            tc.tile_pool(name="outp", bufs=3) as outpp,
            tc.tile_pool(name="ps_proj", bufs=2 if resplit else (3 if colS_burst else 2), space="PSUM") as ps_proj,
            tc.tile_pool(name="ps_sc", bufs=2 if resplit else (3 if colS_burst else 4), space="PSUM") as ps_sc,
            tc.tile_pool(name="ps_s", bufs=2, space="PSUM") as ps_s,
            tc.tile_pool(name="ps_tps", bufs=2, space="PSUM") as ps_tps,
        ):
            # ---------------- constants ----------------
            W = constp.tile([128, ED, 3 * E], f32r)
            wv = w[:].rearrange("(k p) m -> p k m", p=128)
            for k in range(ED):
                for c in range(3):
                    wst = xnp.tile([128, E], f32, tag="xn", name=f"wst{k}_{c}")
                    nc.sync.dma_start(wst, wv[:, k, c * E : (c + 1) * E])
                    nc.vector.tensor_copy(W[:, k, c * E : (c + 1) * E], wst)

            b3 = bvec[:].rearrange("(c m) -> c m", m=E)
            bb = constp.tile([128, 3, E], bf16)
            nc.gpsimd.dma_start(
                bb, bass.AP(tensor=b3.tensor, offset=b3.offset, ap=[[0, 128]] + list(b3.ap))
            )
            bpart = constp.tile([128, 3 * ED], f32)
            nc.gpsimd.dma_start(bpart, bvec[:].rearrange("(c p) -> p c", p=128))

            ident = constp.tile([128, 128], f32)
            make_identity(nc, ident)
            ones8 = constp.tile([128, 2, 1], f8)
            nc.gpsimd.memset(ones8, 1.0)
            if qkt_via_transpose:
                identB = constp.tile([128, 128], bf16)
                make_identity(nc, identB)

            def batch_body():
              for b in range(nb):
                if not slice_xt:
                    xT = bigp.tile([128, ED, N], f32r, tag="xT")
                if qk_merged:
                    QKt = bigp.tile([128, 2, ED, N], f8, tag="QKt",
                                    bufs=2 if slice_xt else 1)
                    Qt, Kt = QKt[:, 0], QKt[:, 1]
                else:
                    Qt = bigp.tile([128, ED, N], f8, tag="Qt")
                    Kt = bigp.tile([128, ED, N], f8, tag="Kt")
                Vn8 = bigp.tile([128, NE, E], f8, tag="Vn8")
                Vnb = bigp.tile([128, NE, E], bf16, tag="Vnb")
                vt_dt = f8 if fp8col else bf16
                Vt = bigp.tile([128, ED, N], vt_dt, tag="Vt",
                               bufs=2 if dbuf_vt else 1)
                Acol = bigp.tile([128, ED, E], bf16, tag="Acol",
                                 bufs=2 if slice_xt else 1)
                AcolT = bigp.tile([128, ED, E], vt_dt, tag="AcolT",
                                  bufs=2 if slice_xt else 1)
                if colS_burst:
                    Qn = bigp.tile([128, NE, E], bf16, tag="Qn")
                    Kn = bigp.tile([128, NE, E], bf16, tag="Kn")
                    scol_ps = None
                else:
                    scol_ps = [
                        ps_sc.tile([128, E], f32, tag="scps", name=f"scol{b}_{i}")
                        for i in range(ED)
                    ]

                # ---- phase A: load x, build xT, projections, col-S accum
                qp = kp = None
                for j in range(NE):
                    s_idx = j // ED
                    xoff, roff = j * 128, s_idx * 512
                    if slice_xt:
                        if j % ED == 0:
                            xT = qkpp.tile(
                                [128, ED, 512], f32r, tag="xsl", name=f"xsl{b}_{s_idx}"
                            )
                        xoff = (j % ED) * 128
                    xn = xnp.tile([128, E], f32, tag="xn")
                    nc.sync.dma_start(xn, x[b, j * 128 : (j + 1) * 128, :])
                    if precast:
                        xnh = xnp.tile([128, E], bf16, tag="xnh")
                        nc.gpsimd.tensor_copy(xnh, xn)
                        xn = xnh
                    t_dt, t_id = (bf16, identB) if precast else (f32, ident)
                    if hw_legal == 1:
                        tpool = ps_tps if resplit else ps_proj
                        tpsx = tpool.tile([128, ED, 128], t_dt, tag="tps" if resplit else "ps", name=f"tpsx{b}_{j}")
                        for k in range(ED):
                            nc.tensor.matmul(
                                tpsx[:, k, :],
                                xn[:, k * 128 : (k + 1) * 128],
                                t_id,
                                is_transpose=True,
                                skip_group_check=True,
                            )
                        nc.vector.tensor_copy(xT[:, :, xoff : xoff + 128], tpsx)
                    elif hw_legal:
                        for k in range(ED):
                            tps = ps_proj.tile([128, 128], f32, tag="ps")
                            nc.tensor.transpose(tps, xn[:, k * 128 : (k + 1) * 128], ident)
                            eng = nc.vector if k % 2 == 0 else nc.scalar
                            if k % 2 == 0:
                                eng.tensor_copy(xT[:, k, xoff : xoff + 128], tps)
                            else:
                                eng.copy(xT[:, k, xoff : xoff + 128], tps)
                    else:
                        for k in range(ED):
                            tps = ps_proj.tile([128, 128], f32, tag="ps")
                            nc.tensor.transpose(tps, xn[:, k * 128 : (k + 1) * 128], ident)
                            nc.gpsimd.tensor_copy(xT[:, k, xoff : xoff + 128], tps)

                    # natural-layout q, k (pair-staged or SBUF-resident), v
                    if colS_burst:
                        qdst, kdst = Qn[:, j, :], Kn[:, j, :]
                    else:
                        if j % 2 == 0:
                            qp = qkpp.tile([128, 2, E], colS_dt, tag="qp", name=f"qp{b}_{j}")
                            kp = qkpp.tile([128, 2, E], colS_dt, tag="kp", name=f"kp{b}_{j}")
                        qdst, kdst = qp[:, j % 2, :], kp[:, j % 2, :]
                    for dst, ci in (
                        (qdst, 0),
                        (kdst, 1),
                        (Vnb[:, j, :], 2),
                    ):
                        pp = ps_proj.tile([128, E], f32, tag="ps")
                        for k in range(ED):
                            nc.tensor.matmul(
                                pp,
                                xT[:, k, xoff : xoff + 128],
                                W[:, k, ci * E : (ci + 1) * E],
                                start=(k == 0),
                                stop=(k == ED - 1),
                            )
                        nc.vector.tensor_add(dst, pp, bb[:, ci, :])
                    nc.gpsimd.tensor_copy(Vn8[:, j, :], Vnb[:, j, :])
                    if not fp8col:
                        vt_eng = nc.sync if colS_burst else nc.scalar
                        vt_eng.dma_start_transpose(
                            Vt[:, :, j * 128 : (j + 1) * 128], Vnb[:, j, :]
                        )

                    # col-S accumulation, every second chunk
                    if not colS_burst and j % 2 == 1:
                        if colS_dt is f8:
                            for i in range(ED):
                                nc.tensor.matmul(
                                    scol_ps[i],
                                    qp[:, :, i * 128 : (i + 1) * 128],
                                    kp,
                                    start=(j == 1),
                                    stop=(j == NE - 1),
                                    perf_mode=DR,
                                )
                        else:
                            for jj in range(2):
                                for i in range(ED):
                                    nc.tensor.matmul(
                                        scol_ps[i],
                                        qp[:, jj, i * 128 : (i + 1) * 128],
                                        kp[:, jj, :],
                                        start=(j == 1 and jj == 0),
                                        stop=(j == NE - 1 and jj == 1),
                                    )

                    # Qt/Kt transposed layouts
                    if qkt_via_transpose and qk_merged:
                        qkpool = ps_tps if resplit else ps_proj
                        psqk = qkpool.tile(
                            [128, 2, ED, 128], bf16, tag="tps" if resplit else "ps", name=f"psqk{b}_{j}"
                        )
                        for ci, src in ((0, qdst), (1, kdst)):
                            for i in range(ED):
                                nc.tensor.matmul(
                                    psqk[:, ci, i, :],
                                    src[:, i * 128 : (i + 1) * 128],
                                    identB,
                                    is_transpose=True,
                                    skip_group_check=True,
                                )
                        nc.vector.tensor_copy(QKt[:, :, :, j * 128 : (j + 1) * 128], psqk)
                    elif qkt_via_transpose:
                        for (dst, src, ci) in ((Qt, qdst, 0), (Kt, kdst, 1)):
                            if hw_legal == 1:
                                psB = ps_proj.tile(
                                    [128, ED, 128], bf16, tag="ps", name=f"psB{b}_{j}_{ci}"
                                )
                                for i in range(ED):
                                    nc.tensor.matmul(
                                        psB[:, i, :],
                                        src[:, i * 128 : (i + 1) * 128],
                                        identB,
                                        is_transpose=True,
                                        skip_group_check=True,
                                    )
                                nc.vector.tensor_copy(
                                    dst[:, :, j * 128 : (j + 1) * 128], psB
                                )
                            else:
                                for i in range(ED):
                                    psB = ps_proj.tile(
                                        [128, 128], bf16, tag="ps", name=f"psB{b}_{j}_{ci}_{i}"
                                    )
                                    nc.tensor.transpose(
                                        psB, src[:, i * 128 : (i + 1) * 128], identB
                                    )
                                    if hw_legal:
                                        if i % 2 == 0:
                                            nc.vector.tensor_copy(
                                                dst[:, i, j * 128 : (j + 1) * 128], psB
                                            )
                                        else:
                                            nc.scalar.copy(
                                                dst[:, i, j * 128 : (j + 1) * 128], psB
                                            )
                                    else:
                                        nc.gpsimd.tensor_copy(
                                            dst[:, i, j * 128 : (j + 1) * 128], psB
                                        )
                    elif j % ED == ED - 1:
                        sl = slice(s_idx * 512, (s_idx + 1) * 512)
                        for dst, ci in ((Qt, 0), (Kt, 1)):
                            for i in range(ED):
                                pp = ps_proj.tile([128, E], f32, tag="ps")
                                for k in range(ED):
                                    nc.tensor.matmul(
                                        pp,
                                        W[:, k, ci * E + i * 128 : ci * E + (i + 1) * 128],
                                        xT[:, k, roff : roff + 512],
                                        start=(k == 0),
                                        stop=(k == ED - 1),
                                    )
                                nc.scalar.activation(
                                    out=dst[:, i, sl],
                                    in_=pp,
                                    func=AF.Identity,
                                    bias=bpart[:, ci * ED + i : ci * ED + i + 1],
                                )

                # ---- phase A2: col softmax (max-sub) + transpose of A
                cstat = statp.tile([128, 3 * ED], f32, tag="cstat")
                for i in range(ED):
                    if colS_burst:
                        scps = ps_sc.tile([128, E], f32, tag="scps", name=f"scol{b}_{i}")
                        for jc in range(NE):
                            nc.tensor.matmul(
                                scps,
                                Qn[:, jc, i * 128 : (i + 1) * 128],
                                Kn[:, jc, :],
                                start=(jc == 0),
                                stop=(jc == NE - 1),
                            )
                    else:
                        scps = scol_ps[i]
                    nm = cstat[:, 2 * ED + i : 2 * ED + i + 1]
                    nc.vector.reduce_max(nm, scps, axis=AX.X, negate=True)
                    nc.vector.tensor_scalar_mul(nm, nm, SCALE)
                    nc.scalar.activation(
                        out=Acol[:, i, :],
                        in_=scps,
                        func=AF.Exp,
                        scale=SCALE,
                        bias=nm,
                        accum_out=cstat[:, i : i + 1],
                    )
                nc.vector.reciprocal(cstat[:, ED : 2 * ED], cstat[:, 0:ED])
                norm_eng = nc.gpsimd if hw_legal else nc.vector
                for i in range(ED):
                    norm_eng.tensor_scalar_mul(
                        Acol[:, i, :], Acol[:, i, :], cstat[:, ED + i : ED + i + 1]
                    )
                    if not fp8col:
                        nc.scalar.dma_start_transpose(
                            AcolT[:, :, i * 128 : (i + 1) * 128], Acol[:, i, :]
                        )

                # ---- phase B: St-direct row attention + merged output
                for s in range(ED):
                    ssl = slice(s * 512, (s + 1) * 512)
                    StE = stepp.tile([128, NE, 512], f8, tag="ste", name=f"ste{b}_{s}")
                    for m in range(NE):
                        sps = ps_s.tile([128, 512], f32, tag="s")
                        for kk in range(2):
                            nc.tensor.matmul(
                                sps,
                                Kt[:, 2 * kk : 2 * kk + 2, m * 128 : (m + 1) * 128],
                                Qt[:, 2 * kk : 2 * kk + 2, ssl],
                                start=(kk == 0),
                                stop=(kk == 1),
                                perf_mode=DR,
                            )
                        nc.scalar.activation(
                            out=StE[:, m, :], in_=sps, func=AF.Exp, scale=SCALE
                        )
                    rs_ps = ps_sc.tile([128, 4], f32, tag="scps", name=f"rs{b}_{s}")
                    for jj in range(4):
                        for mm in range(8):
                            nc.tensor.matmul(
                                rs_ps[:, jj : jj + 1],
                                StE[:, 2 * mm : 2 * mm + 2, jj * 128 : (jj + 1) * 128],
                                ones8,
                                start=(mm == 0),
                                stop=(mm == 7),
                                perf_mode=DR,
                                skip_group_check=True,
                            )
                    rstat = statp.tile([128, 4], f32, tag="rstat")
                    nc.vector.reciprocal(rstat, rs_ps)
                    for jj in range(4):
                        j = s * 4 + jj
                        jsl = slice(j * 128, (j + 1) * 128)
                        po = ps_sc.tile([128, E], f32, tag="scps")
                        for mm in range(8):
                            nc.tensor.matmul(
                                po,
                                StE[:, 2 * mm : 2 * mm + 2, jj * 128 : (jj + 1) * 128],
                                Vn8[:, 2 * mm : 2 * mm + 2, :],
                                start=(mm == 0),
                                stop=(mm == 7),
                                perf_mode=DR,
                            )
                        oc = ps_sc.tile([128, E], f32, tag="scps")
                        if fp8col:
                            for c in range(2):
                                nc.tensor.matmul(
                                    oc,
                                    Vt[:, 2 * c : 2 * c + 2, jsl],
                                    AcolT[:, 2 * c : 2 * c + 2, :],
                                    start=(c == 0),
                                    stop=(c == 1),
                                    perf_mode=DR,
                                )
                        else:
                            for c in range(ED):
                                nc.tensor.matmul(
                                    oc,
                                    Vt[:, c, jsl],
                                    AcolT[:, c, :],
                                    start=(c == 0),
                                    stop=(c == ED - 1),
                                )
                        octmp = outpp.tile([128, E], f32, tag="octmp")
                        if qk_merged:
                            nc.scalar.copy(octmp, oc)
                        elif hw_legal:
                            nc.vector.tensor_copy(octmp, oc)
                        else:
                            nc.gpsimd.tensor_copy(octmp, oc)
                        ot = outpp.tile([128, E], f32, tag="ot")
                        nc.vector.scalar_tensor_tensor(
                            ot,
                            po,
                            rstat[:, jj : jj + 1],
                            octmp,
                            op0=mybir.AluOpType.mult,
                            op1=mybir.AluOpType.add,
                        )
                        nc.sync.dma_start(y[b, jsl, :], ot)

            if reps == 1:
                batch_body()
            else:
                with tc.For_i(0, reps, 1):
                    batch_body()

    nc.compile()
    return nc


BEST_VARIANT = 36


def kernel(x, w_qkv, b_qkv):
    from concourse.bass_utils import run_bass_kernel_spmd

    nc = build_nc(NB, BEST_VARIANT)
    in_maps = make_in_maps(x, w_qkv, b_qkv)
    res = run_bass_kernel_spmd(nc, in_maps, core_ids=list(range(NCORES)))
    out = np.empty((B, N, E), dtype=np.float32)
    for c in range(NCORES):
        out[c * NB : (c + 1) * NB] = res.results[c]["y"]
    return out

